# revision 11
# baseline (speedup 1.0000x reference)
"""MoE-routing attention kernel for 8 Trainium2 NeuronCores — v2.

Expert parallelism (1 expert per core), full inputs in, full output out.
v2 restructures v1 around three ideas:

1. Merged q/k projection: S[s,t] = q_t.k_s = x_t M x_s + a[t] + c[s] + cc
   with M = Wq Wk^T precomputed ON DEVICE once per expert (dense 1024^3
   matmul that also warms the PE), a = x.(Wq bk), c = x.(Wk bq),
   cc = bq.bk. Per batch this is ONE projection G' = x_g M^T (64 MMs)
   plus 24 score MMs instead of two projections (128 MMs) + scores.
2. bf16 everywhere on the S path (validated: rel err stays at 1e-6
   because scores are divided by D=1024 before exp). bf16 halves
   LDWEIGHTS cost (FWL) and DMA volume.
3. Gathered x arrives TRANSPOSED directly via dma_gather(transpose=True)
   (bf16, one instruction per batch) — no PE transposes, no PSUM copies.
   Pad slots point at a host-appended zero row of x.

The gate runs with wg STATIONARY (8 LDWEIGHTS total instead of 256):
logits^T = wg^T @ x^T in [8, N] layout, then 8 tiny PE transposes per
batch back to [token, 8] for the top-2/softmax tail (fp32 - routing
flips are the one numerically fragile spot).

Weight-derived vectors (u = wv @ wo_rowsum, aw = Wq bk, cv = Wk bq and
scalars cc, c0 = bv.wo_rowsum, boS = sum bo) are folded on the host like
bias preprocessing. All model FLOPs (gate, M, G', S, attention combine)
stay on device.

Host: sums the 8 per-core [B,T] contribution vectors and applies the
final log_softmax (the SPMD combine/unshard step), as in v1.
"""

import sys

import numpy as np

for _p in ("/opt/trn_rl_repo", "/root/.axon_site/_ro/trn_rl_repo"):
    if _p not in sys.path:
        sys.path.append(_p)

import ml_dtypes  # noqa: E402

import concourse.bass as bass  # noqa: E402
import concourse.bass_isa as bass_isa  # noqa: E402
import concourse.mybir as mybir  # noqa: E402
import concourse.tile as tile  # noqa: E402
from concourse import bacc  # noqa: E402
from concourse import bass_utils  # noqa: E402
from concourse.bass import ts  # noqa: E402
from concourse.masks import make_identity  # noqa: E402

P = 128
B, T, D, E = 4, 1024, 1024, 8
DH = D
N = B * T
DC = D // P  # 8 contraction chunks
ST = T // P  # 8 token tiles per batch
CAP = 384  # gathered slot capacity per (expert, batch)
SC = CAP // P  # 3 slot tiles
NZ = N  # index of the host-appended all-zero x row (pad target)
BIG = 1 << 20
F32 = mybir.dt.float32
F32R = mybir.dt.float32r
BF16 = mybir.dt.bfloat16
I32 = mybir.dt.int32
I16 = mybir.dt.int16
AF = mybir.ActivationFunctionType
OP = mybir.AluOpType
AX = mybir.AxisListType
RED = bass_isa.ReduceOp

_CACHE = {}


def _emit(nc, tc, dt_in, dt_out):
    (xT, xnb, wg_d, wqT_d, wkT_d, w3_d, sconst_d) = dt_in
    (out_d,) = dt_out

    with tc.tile_pool(name="const", bufs=1) as const, tc.tile_pool(
        name="wqk", bufs=1
    ) as wqk, tc.tile_pool(name="msb", bufs=1) as msbp, tc.tile_pool(
        name="drams", bufs=1, space="DRAM"
    ) as dramp:
        # ---------------- constants ----------------
        wg_sb = const.tile([P, DC, E], F32)
        nc.sync.dma_start(wg_sb[:], wg_d.ap().rearrange("(c p) e -> p c e", p=P))
        w3_sb = const.tile([P, DC, 3], BF16)
        nc.sync.dma_start(w3_sb[:], w3_d.ap().rearrange("(c p) k -> p c k", p=P))
        bias3 = const.tile([3, 1], F32)
        nc.sync.dma_start(bias3[:], sconst_d.ap()[0:3, None])
        boS_bc = const.tile([P, 1], F32)
        nc.sync.dma_start(
            boS_bc[:], sconst_d.ap()[3:4][None, :].to_broadcast([P, 1])
        )

        idn = const.tile([P, P], F32)
        make_identity(nc, idn[:])
        ones_bf = const.tile([1, P], BF16)
        nc.vector.memset(ones_bf[:], 1.0)
        iota_f_i = const.tile([P, P], I32)
        nc.gpsimd.iota(iota_f_i[:], pattern=[[1, P]], base=0, channel_multiplier=0)
        iota_p_i = const.tile([P, 1], I32)
        nc.gpsimd.iota(iota_p_i[:], pattern=[[0, 1]], base=0, channel_multiplier=1)
        iota_ff = const.tile([P, P], F32)
        nc.vector.tensor_copy(iota_ff[:], iota_f_i[:])
        iota_pf = const.tile([P, 1], F32)
        nc.vector.tensor_copy(iota_pf[:], iota_p_i[:])
        ltri = const.tile([P, P], F32)  # ltri[k, m] = (m > k)
        nc.vector.tensor_scalar(ltri[:], iota_ff[:], iota_pf[:], None, op0=OP.is_gt)
        iosc_i = const.tile([P, SC], I32)  # value = slot j = c*128 + p
        nc.gpsimd.iota(iosc_i[:], pattern=[[P, SC]], base=0, channel_multiplier=1)
        iosc = const.tile([P, SC], F32)
        nc.vector.tensor_copy(iosc[:], iosc_i[:])
        tv8 = const.tile([P, ST], I32)  # within-batch token id t = c*128 + p
        nc.gpsimd.iota(tv8[:], pattern=[[P, ST]], base=0, channel_multiplier=1)
        nzt = const.tile([P, SC], I32)
        nc.vector.memset(nzt[:], NZ)
        zt = const.tile([P, ST], F32)
        nc.vector.memset(zt[:], 0.0)

        Msb = msbp.tile([P, DC, D], BF16)  # M^T[d', d]; chunk dc = d' rows

        sc_d = dramp.tile([N], F32, tag="scd", name="scd")
        idx_d = [
            dramp.tile([CAP], I32, tag=f"idxd{b}", name=f"idxd{b}")
            for b in range(B)
        ]
        cu_d = [
            dramp.tile([2, CAP], F32, tag=f"cud{b}", name=f"cud{b}")
            for b in range(B)
        ]

        with tc.tile_pool(name="pb", bufs=1) as pbp, tc.tile_pool(
            name="xt", bufs=2
        ) as xtp, tc.tile_pool(name="gsb", bufs=3) as gsb, tc.tile_pool(
            name="logt", bufs=2
        ) as logtp, tc.tile_pool(name="gbig", bufs=2) as gbig, tc.tile_pool(
            name="psm", bufs=1, space="PSUM"
        ) as psm, tc.tile_pool(name="psg", bufs=1, space="PSUM") as psg:
            # per-batch persistent tiles
            maskb = [
                pbp.tile([P, ST], F32, tag=f"maskb{b}", name=f"maskb{b}")
                for b in range(B)
            ]
            cwb = [
                pbp.tile([P, ST], F32, tag=f"cwb{b}", name=f"cwb{b}")
                for b in range(B)
            ]
            idxt = [
                pbp.tile([P, SC], I32, tag=f"idxt{b}", name=f"idxt{b}")
                for b in range(B)
            ]
            idx16 = [
                pbp.tile([P, CAP // 16], I16, tag=f"idx16{b}", name=f"idx16{b}")
                for b in range(B)
            ]
            xgT = [
                pbp.tile([P, DC, CAP], BF16, tag=f"xgT{b}", name=f"xgT{b}")
                for b in range(B)
            ]
            omc = [
                pbp.tile([P, SC], F32, tag=f"omc{b}", name=f"omc{b}")
                for b in range(B)
            ]
            cspd = [
                pbp.tile([P, SC], F32, tag=f"cspd{b}", name=f"cspd{b}")
                for b in range(B)
            ]
            wv_w = [
                pbp.tile([P, SC], BF16, tag=f"wvw{b}", name=f"wvw{b}")
                for b in range(B)
            ]
            a_bf = [
                pbp.tile([1, CAP], BF16, tag=f"abf{b}", name=f"abf{b}")
                for b in range(B)
            ]

            def m_build(fcs):
                """M^T[d'-tile, d] = sum_f WkT[f, d']^T WqT[f, d] for d'-tiles
                in fcs. Each group accumulates over all 8 f chunks."""
                for dt in fcs:
                    for half in range(2):
                        pm = psm.tile(
                            [P, 512], F32, tag="m", bufs=2,
                            name=f"m{dt}_{half}",
                        )
                        for fc in range(DC):
                            nc.tensor.matmul(
                                pm[:],
                                wk_sb[:, fc, ts(dt, P)],
                                wq_sb[:, fc, ts(half, 512)],
                                start=(fc == 0),
                                stop=(fc == DC - 1),
                            )
                        nc.scalar.activation(
                            Msb[:, dt, ts(half, 512)], pm[:], AF.Copy
                        )

            def gate(b):
                xt = xtp.tile([P, DC, T], F32, tag="xt", name=f"xt{b}")
                nc.sync.dma_start(
                    xt[:],
                    xT.ap().rearrange("(c p) n -> p c n", p=P)[
                        :, :, b * T : (b + 1) * T
                    ],
                )
                logT = logtp.tile([8, T], F32, tag="logT", name=f"logT{b}")
                for half in range(2):
                    pg = psm.tile(
                        [P, 512], F32, tag="m", bufs=2, name=f"g{b}_{half}"
                    )
                    for dc in range(DC):
                        nc.tensor.matmul(
                            pg[0:E, :],
                            wg_sb[:, dc],
                            xt[:, dc, ts(half, 512)],
                            start=(dc == 0),
                            stop=(dc == DC - 1),
                        )
                    nc.scalar.activation(logT[:, ts(half, 512)], pg[0:E, :], AF.Copy)
                for tt in range(ST):
                    tp = psg.tile(
                        [P, 512], F32, tag="g", bufs=3, name=f"tp{b}_{tt}"
                    )
                    nc.tensor.transpose(
                        tp[:, 0:E], logT[:, ts(tt, P)], idn[0:E, 0:E]
                    )
                    gl = gsb.tile([P, E], F32, tag="gl")
                    nc.scalar.activation(gl[:], tp[:, 0:E], AF.Copy)
                    mx8 = gsb.tile([P, 8], F32, tag="mx8")
                    nc.vector.max(out=mx8[:], in_=gl[:])
                    mxn = gsb.tile([P, 1], F32, tag="mxn")
                    nc.vector.tensor_scalar_mul(mxn[:], mx8[:, 0:1], -1.0)
                    probs = gsb.tile([P, E], F32, tag="probs")
                    se = gsb.tile([P, 1], F32, tag="se")
                    nc.scalar.activation(
                        probs[:], gl[:], AF.Exp, bias=mxn[:], scale=1.0,
                        accum_out=se[:],
                    )
                    rs = gsb.tile([P, 1], F32, tag="rs")
                    nc.vector.reciprocal(rs[:], se[:])
                    nc.vector.tensor_scalar(
                        maskb[b][:, tt : tt + 1], gl[:, 0:1], mx8[:, 1:2], None,
                        op0=OP.is_ge,
                    )
                    nc.vector.scalar_tensor_tensor(
                        cwb[b][:, tt : tt + 1],
                        probs[:, 0:1],
                        rs[:],
                        maskb[b][:, tt : tt + 1],
                        op0=OP.mult,
                        op1=OP.mult,
                    )

            def gather(b):
                """Prefix-sum slot assignment, index build, transposed bf16
                x gather, omega weights."""
                tot = gsb.tile([P, ST], F32, tag="tot")
                nc.gpsimd.partition_all_reduce(
                    tot[:], maskb[b][:], channels=P, reduce_op=RED.add
                )
                carry = gsb.tile([P, ST], F32, tag="carry")
                nc.vector.memset(carry[:, 0:1], 0.0)
                for tt in range(1, ST):
                    nc.vector.tensor_tensor(
                        carry[:, tt : tt + 1],
                        carry[:, tt - 1 : tt],
                        tot[:, tt - 1 : tt],
                        op=OP.add,
                    )
                cf = gsb.tile([P, 1], F32, tag="cf")  # total count C
                nc.vector.tensor_tensor(
                    cf[:], carry[:, ST - 1 : ST], tot[:, ST - 1 : ST], op=OP.add
                )
                tvb = gsb.tile([P, ST], I32, tag="tvb")
                nc.vector.tensor_scalar(tvb[:], tv8[:], b * T, None, op0=OP.add)
                nc.sync.dma_start(
                    idx_d[b].rearrange("(c p) -> p c", p=P), nzt[:]
                )
                # slot position per token: exclusive prefix within tile
                # (ltri matmul) + carry; unrouted tokens pushed out of range
                pp = psg.tile([P, 512], F32, tag="g", bufs=3, name=f"pp{b}")
                nc.tensor.matmul(
                    pp[:, 0:ST], ltri[:], maskb[b][:], start=True, stop=True
                )
                gp = gsb.tile([P, ST], F32, tag="gp")
                nc.vector.tensor_tensor(gp[:], pp[:, 0:ST], carry[:], op=OP.add)
                gm = gsb.tile([P, ST], F32, tag="gm")
                nc.vector.tensor_scalar(
                    gm[:], maskb[b][:], float(-BIG), float(BIG),
                    op0=OP.mult, op1=OP.add,
                )
                nc.vector.tensor_add(gm[:], gm[:], gp[:])
                gposi = gsb.tile([P, ST], I32, tag="gposi")
                nc.vector.tensor_copy(gposi[:], gm[:])
                for tt in range(ST):
                    nc.gpsimd.indirect_dma_start(
                        out=idx_d[b][:, None],
                        out_offset=bass.IndirectOffsetOnAxis(
                            ap=gposi[:, tt : tt + 1], axis=0
                        ),
                        in_=tvb[:, tt : tt + 1],
                        in_offset=None,
                        bounds_check=CAP - 1,
                        oob_is_err=False,
                    )
                # read back: int32 slot->token for the output scatter, and
                # int16 lane-wrapped copy for dma_gather
                nc.sync.dma_start(
                    idxt[b][:], idx_d[b].rearrange("(c p) -> p c", p=P)
                )
                for r in range(P // 16):
                    nc.gpsimd.dma_start(
                        idx16[b][16 * r : 16 * (r + 1), :],
                        idx_d[b].rearrange("(c p) -> p c", p=16),
                    )
                nc.gpsimd.dma_gather(
                    xgT[b][:],
                    xnb.ap(),
                    idx16[b][:],
                    num_idxs=CAP,
                    num_idxs_reg=CAP,
                    elem_size=D,
                    transpose=True,
                )
                # omega: 1 for j < C, (T - C) at j == CAP-1, else 0
                tmc = gsb.tile([P, 1], F32, tag="tmc")
                nc.vector.tensor_scalar(
                    tmc[:], cf[:], -1.0, float(T), op0=OP.mult, op1=OP.add
                )
                rep = gsb.tile([P, SC], F32, tag="rep")
                nc.vector.tensor_scalar(
                    omc[b][:], iosc[:], cf[:], None, op0=OP.is_lt
                )
                nc.vector.tensor_scalar(
                    rep[:], iosc[:], float(CAP - 1), None, op0=OP.is_equal
                )
                nc.vector.tensor_scalar(rep[:], rep[:], tmc[:], None, op0=OP.mult)
                nc.vector.tensor_add(omc[b][:], omc[b][:], rep[:])

                # --- a, c, vw vectors: [3, CAP] = w3^T x_g^T (+ biases) ---
                p3 = psg.tile([P, 512], F32, tag="g", bufs=3, name=f"p3{b}")
                for dc in range(DC):
                    nc.tensor.matmul(
                        p3[0:3, 0:CAP],
                        w3_sb[:, dc],
                        xgT[b][:, dc],
                        start=(dc == 0),
                        stop=(dc == DC - 1),
                    )
                avc = gsb.tile([3, CAP], F32, tag="avc")
                nc.scalar.activation(
                    avc[:], p3[0:3, 0:CAP], AF.Identity, bias=bias3[:]
                )
                nc.vector.tensor_copy(a_bf[b][:], avc[0:1, :])
                nc.sync.dma_start(cu_d[b][:], avc[1:3, :])
                nc.sync.dma_start(
                    cspd[b][:], cu_d[b][0].rearrange("(c p) -> p c", p=P)
                )
                vwsp = gsb.tile([P, SC], F32, tag="vwsp")
                nc.sync.dma_start(
                    vwsp[:], cu_d[b][1].rearrange("(c p) -> p c", p=P)
                )
                nc.vector.tensor_scalar_mul(cspd[b][:], cspd[b][:], 1.0 / D)
                vww = gsb.tile([P, SC], F32, tag="vww")
                nc.vector.tensor_mul(vww[:], vwsp[:], omc[b][:])
                nc.vector.tensor_copy(wv_w[b][:], vww[:])

            def block(b):
                """Per-batch dense compute: G' = x_g M^T, S, exp, Z, num,
                combine."""
                # --- G'^T[d, s] = sum_d' M^T[d', d] xgT[d', s] ---
                Gsb = gbig.tile([P, DC, CAP], BF16, tag="Gsb", name=f"Gsb{b}")
                for dt in range(DC):
                    pgt = psg.tile(
                        [P, 512], F32, tag="g", bufs=3, name=f"G{b}_{dt}"
                    )
                    for dc in range(DC):
                        nc.tensor.matmul(
                            pgt[:, 0:CAP],
                            Msb[:, dc, ts(dt, P)],
                            xgT[b][:, dc],
                            start=(dc == 0),
                            stop=(dc == DC - 1),
                        )
                    nc.scalar.activation(Gsb[:, dt], pgt[:, 0:CAP], AF.Copy)

                # --- S[s, t] = sum_d G'^T[d, s] xgT[d, t] + a[t] (+c[s]/D
                # via exp bias); E = exp(S/D) with free-axis row sums ---
                Eg = gbig.tile([P, SC, CAP], BF16, tag="Eg", name=f"Eg{b}")
                eacc = gsb.tile([P, SC], F32, tag="eacc")
                erw = gsb.tile([P, SC], F32, tag="erw")
                for st in range(SC):
                    pss = psg.tile(
                        [P, 512], F32, tag="g", bufs=3, name=f"S{b}_{st}"
                    )
                    for dt in range(DC):
                        nc.tensor.matmul(
                            pss[:, 0:CAP],
                            Gsb[:, dt, ts(st, P)],
                            xgT[b][:, dt],
                            start=(dt == 0),
                            stop=False,
                        )
                    nc.tensor.matmul(
                        pss[:, 0:CAP],
                        ones_bf[:],
                        a_bf[b][:],
                        start=False,
                        stop=True,
                    )
                    nc.scalar.activation(
                        Eg[:, st], pss[:, 0:CAP], AF.Exp,
                        bias=cspd[b][:, st : st + 1], scale=float(1.0 / D),
                        accum_out=eacc[:, st : st + 1],
                    )
                    nc.vector.tensor_scalar(
                        erw[:, st : st + 1],
                        Eg[:, st, CAP - 1 : CAP],
                        float(T - CAP),
                        eacc[:, st : st + 1],
                        op0=OP.mult,
                        op1=OP.add,
                    )
                # Z = omega . erw (both [slot] in partition-chunk layout)
                scr3 = gsb.tile([P, SC], F32, tag="scr3")
                zp = gsb.tile([P, 1], F32, tag="zp")
                nc.vector.tensor_mul(scr3[:], erw[:], omc[b][:])
                nc.vector.reduce_sum(zp[:], scr3[:], axis=AX.X)
                za = gsb.tile([P, 1], F32, tag="za")
                nc.gpsimd.partition_all_reduce(
                    za[:], zp[:], channels=P, reduce_op=RED.add
                )
                rZ = gsb.tile([P, 1], F32, tag="rZ")
                nc.vector.reciprocal(rZ[:], za[:])
                # num[t] = sum_s E[s, t] (omega_s vw[s])
                numg = gsb.tile([P, SC], F32, tag="numg")
                for ti in range(SC):
                    pn = psg.tile(
                        [P, 512], F32, tag="g", bufs=3, name=f"pn{b}_{ti}"
                    )
                    for sc in range(SC):
                        nc.tensor.matmul(
                            pn[:, 0:1],
                            Eg[:, sc, ts(ti, P)],
                            wv_w[b][:, sc : sc + 1],
                            start=(sc == 0),
                            stop=(sc == SC - 1),
                        )
                    nc.scalar.activation(numg[:, ti : ti + 1], pn[:, 0:1], AF.Copy)
                # out_sum = num/Z + boS; scatter to token space
                outg = gsb.tile([P, SC], F32, tag="outg")
                nc.vector.tensor_scalar(
                    outg[:], numg[:], rZ[:], boS_bc[:], op0=OP.mult, op1=OP.add
                )
                for i in range(SC):
                    nc.gpsimd.indirect_dma_start(
                        out=sc_d[:, None],
                        out_offset=bass.IndirectOffsetOnAxis(
                            ap=idxt[b][:, i : i + 1], axis=0
                        ),
                        in_=outg[:, i : i + 1],
                        in_offset=None,
                        bounds_check=N - 1,
                        oob_is_err=False,
                    )
                scb = gsb.tile([P, ST], F32, tag="scb")
                nc.sync.dma_start(
                    scb[:],
                    sc_d[b * T : (b + 1) * T].rearrange("(c p) -> p c", p=P),
                )
                ob = gsb.tile([P, ST], F32, tag="ob")
                nc.vector.tensor_mul(ob[:], scb[:], cwb[b][:])
                nc.sync.dma_start(out_d.ap()[b], ob[:])

            # ---------------- pipeline ----------------
            wq_sb = wqk.tile([P, DC, D], BF16, tag="wq", name="wq")
            wk_sb = wqk.tile([P, DC, D], BF16, tag="wk", name="wk")
            nc.sync.dma_start(
                wq_sb[:], wqT_d.ap().rearrange("(c p) d -> p c d", p=P)
            )
            nc.sync.dma_start(
                wk_sb[:], wkT_d.ap().rearrange("(c p) d -> p c d", p=P)
            )
            # zero the token-space scatter target once
            for bb in range(B):
                nc.sync.dma_start(
                    sc_d[bb * T : (bb + 1) * T].rearrange("(c p) -> p c", p=P),
                    zt[:],
                )
            m_build(range(DC))
            gate(0)
            gather(0)
            gate(1)
            gather(1)
            gate(2)
            gather(2)
            gate(3)
            gather(3)
            block(0)
            block(1)
            block(2)
            block(3)


def build_nc():
    nc = bacc.Bacc("TRN2", target_bir_lowering=False, debug=False, num_devices=8)
    xT = nc.dram_tensor("xT", [D, N], F32, kind="ExternalInput")
    xnb = nc.dram_tensor("xnb", [N + 1, D], BF16, kind="ExternalInput")
    wg_d = nc.dram_tensor("wg", [D, E], F32, kind="ExternalInput")
    wqT_d = nc.dram_tensor("wqT", [DH, D], BF16, kind="ExternalInput")
    wkT_d = nc.dram_tensor("wkT", [DH, D], BF16, kind="ExternalInput")
    w3_d = nc.dram_tensor("w3", [D, 3], BF16, kind="ExternalInput")
    sconst_d = nc.dram_tensor("sconst", [8], F32, kind="ExternalInput")
    out_d = nc.dram_tensor("contrib", [B, P, ST], F32, kind="ExternalOutput")
    with tile.TileContext(nc) as tc:
        _emit(
            nc,
            tc,
            (xT, xnb, wg_d, wqT_d, wkT_d, w3_d, sconst_d),
            (out_d,),
        )
    nc.compile()
    return nc


def make_in_maps(x, wg, wqkv, bqkv, wo, bo):
    xn = np.ascontiguousarray(x.reshape(N, D), dtype=np.float32)
    xT = np.ascontiguousarray(xn.T)
    xnb = np.zeros((N + 1, D), dtype=ml_dtypes.bfloat16)
    xnb[:N] = xn
    in_maps = []
    for e in range(E):
        perm = [e] + [j for j in range(E) if j != e]
        wq = wqkv[e][:, 0::3].astype(np.float64)
        wk = wqkv[e][:, 1::3].astype(np.float64)
        wv = wqkv[e][:, 2::3].astype(np.float64)
        bq = bqkv[e][0::3].astype(np.float64)
        bk = bqkv[e][1::3].astype(np.float64)
        bv = bqkv[e][2::3].astype(np.float64)
        wos = wo[e].astype(np.float64).sum(axis=1)
        u = wv @ wos
        aw = wq @ bk
        cv = wk @ bq
        cc = float(bq @ bk)
        c0 = float(bv @ wos)
        boS = float(bo[e].astype(np.float64).sum())
        in_maps.append(
            {
                "xT": xT,
                "xnb": xnb,
                "wg": np.ascontiguousarray(wg[:, perm], dtype=np.float32),
                "wqT": np.ascontiguousarray(wq.T).astype(ml_dtypes.bfloat16),
                "wkT": np.ascontiguousarray(wk.T).astype(ml_dtypes.bfloat16),
                "w3": np.ascontiguousarray(
                    np.stack([aw, cv, u], axis=1)
                ).astype(ml_dtypes.bfloat16),
                "sconst": np.array(
                    [cc, 0.0, c0, boS, 0.0, 0.0, 0.0, 0.0], dtype=np.float32
                ),
            }
        )
    return in_maps


def run_device(in_maps, trace=False):
    if "nc" not in _CACHE:
        _CACHE["nc"] = build_nc()
    return bass_utils.run_bass_kernel_spmd(
        _CACHE["nc"], in_maps, core_ids=list(range(E)), trace=trace
    )


def kernel(x, wg, wqkv, bqkv, wo, bo, top_k):
    assert int(top_k) == 2, f"kernel hardcodes top_k=2, got {top_k}"
    x = np.asarray(x, np.float32)
    wg = np.asarray(wg, np.float32)
    wqkv = np.asarray(wqkv, np.float32)
    bqkv = np.asarray(bqkv, np.float32)
    wo = np.asarray(wo, np.float32)
    bo = np.asarray(bo, np.float32)

    res = run_device(make_in_maps(x, wg, wqkv, bqkv, wo, bo))
    total = np.zeros((B, T), np.float64)
    for c in range(E):
        contrib = res.results[c]["contrib"]  # [B, P, ST], t = tt*128 + p
        total += contrib.transpose(0, 2, 1).reshape(B, T).astype(np.float64)
    m = total.max(axis=1, keepdims=True)
    ls = total - m - np.log(np.exp(total - m).sum(axis=1, keepdims=True))
    return ls.astype(np.float32)


# revision 13
# speedup vs baseline: 1.2220x; 1.2220x over previous
"""MoE-routing attention kernel for 8 Trainium2 NeuronCores — v2.

Expert parallelism (1 expert per core), full inputs in, full output out.
v2 restructures v1 around three ideas:

1. Merged q/k projection: S[s,t] = q_t.k_s = x_t M x_s + a[t] + c[s] + cc
   with M = Wq Wk^T precomputed ON DEVICE once per expert (dense 1024^3
   matmul that also warms the PE), a = x.(Wq bk), c = x.(Wk bq),
   cc = bq.bk. Per batch this is ONE projection G' = x_g M^T (64 MMs)
   plus 24 score MMs instead of two projections (128 MMs) + scores.
2. bf16 everywhere on the S path (validated: rel err stays at 1e-6
   because scores are divided by D=1024 before exp). bf16 halves
   LDWEIGHTS cost (FWL) and DMA volume.
3. Gathered x arrives TRANSPOSED directly via dma_gather(transpose=True)
   (bf16, one instruction per batch) — no PE transposes, no PSUM copies.
   Pad slots point at a host-appended zero row of x.

The gate runs with wg STATIONARY (8 LDWEIGHTS total instead of 256):
logits^T = wg^T @ x^T in [8, N] layout, then 8 tiny PE transposes per
batch back to [token, 8] for the top-2/softmax tail (fp32 - routing
flips are the one numerically fragile spot).

Weight-derived vectors (u = wv @ wo_rowsum, aw = Wq bk, cv = Wk bq and
scalars cc, c0 = bv.wo_rowsum, boS = sum bo) are folded on the host like
bias preprocessing. All model FLOPs (gate, M, G', S, attention combine)
stay on device.

Host: sums the 8 per-core [B,T] contribution vectors and applies the
final log_softmax (the SPMD combine/unshard step), as in v1.
"""

import sys

import numpy as np

for _p in ("/opt/trn_rl_repo", "/root/.axon_site/_ro/trn_rl_repo"):
    if _p not in sys.path:
        sys.path.append(_p)

import ml_dtypes  # noqa: E402

import concourse.bass as bass  # noqa: E402
import concourse.bass_isa as bass_isa  # noqa: E402
import concourse.mybir as mybir  # noqa: E402
import concourse.tile as tile  # noqa: E402
from concourse import bacc  # noqa: E402
from concourse import bass_utils  # noqa: E402
from concourse.bass import ts  # noqa: E402
from concourse.masks import make_identity  # noqa: E402

P = 128
B, T, D, E = 4, 1024, 1024, 8
DH = D
N = B * T
DC = D // P  # 8 contraction chunks
ST = T // P  # 8 token tiles per batch
CAP = 384  # gathered slot capacity per (expert, batch)
SC = CAP // P  # 3 slot tiles
NZ = N  # index of the host-appended all-zero x row (pad target)
BIG = 1 << 20
F32 = mybir.dt.float32
F32R = mybir.dt.float32r
BF16 = mybir.dt.bfloat16
I32 = mybir.dt.int32
I16 = mybir.dt.int16
AF = mybir.ActivationFunctionType
OP = mybir.AluOpType
AX = mybir.AxisListType
RED = bass_isa.ReduceOp

_CACHE = {}


def _emit(nc, tc, dt_in, dt_out):
    (xT, xnb, wg_d, wqT_d, wkT_d, w3_d, sconst_d) = dt_in
    (out_d,) = dt_out

    with tc.tile_pool(name="const", bufs=1) as const, tc.tile_pool(
        name="wqk", bufs=1
    ) as wqk, tc.tile_pool(name="msb", bufs=1) as msbp, tc.tile_pool(
        name="drams", bufs=1, space="DRAM"
    ) as dramp:
        # ---------------- constants ----------------
        wg_sb = const.tile([P, DC, E], F32)
        nc.sync.dma_start(wg_sb[:], wg_d.ap().rearrange("(c p) e -> p c e", p=P))
        w3_sb = const.tile([P, DC, 3], BF16)
        nc.sync.dma_start(w3_sb[:], w3_d.ap().rearrange("(c p) k -> p c k", p=P))
        bias3 = const.tile([3, 1], F32)
        nc.sync.dma_start(bias3[:], sconst_d.ap()[0:3, None])
        boS_bc = const.tile([P, 1], F32)
        nc.sync.dma_start(
            boS_bc[:], sconst_d.ap()[3:4][None, :].to_broadcast([P, 1])
        )

        idn = const.tile([P, P], F32)
        make_identity(nc, idn[:])
        ones_bf = const.tile([1, P], BF16)
        nc.vector.memset(ones_bf[:], 1.0)
        iosc_i = const.tile([P, SC], I32)  # value = slot j = c*128 + p
        nc.gpsimd.iota(iosc_i[:], pattern=[[P, SC]], base=0, channel_multiplier=1)
        iosc = const.tile([P, SC], F32)
        nc.vector.tensor_copy(iosc[:], iosc_i[:])
        tv8 = const.tile([P, ST], I32)  # within-batch token id t = c*128 + p
        nc.gpsimd.iota(tv8[:], pattern=[[P, ST]], base=0, channel_multiplier=1)
        tvf = const.tile([P, ST], F32)
        nc.vector.tensor_copy(tvf[:], tv8[:])
        zt = const.tile([P, ST], F32)
        nc.vector.memset(zt[:], 0.0)

        Msb = msbp.tile([P, DC, D], BF16)  # M^T[d', d]; chunk dc = d' rows

        sc_d = dramp.tile([N], F32, tag="scd", name="scd")
        mb_d = [
            dramp.tile([T], F32, tag=f"mbd{b}", name=f"mbd{b}")
            for b in range(B)
        ]
        iw_d = [
            dramp.tile([CAP], F32, tag=f"iwd{b}", name=f"iwd{b}")
            for b in range(B)
        ]
        cu_d = [
            dramp.tile([2, CAP], F32, tag=f"cud{b}", name=f"cud{b}")
            for b in range(B)
        ]

        with tc.tile_pool(name="pb", bufs=1) as pbp, tc.tile_pool(
            name="xt", bufs=2
        ) as xtp, tc.tile_pool(name="gsb", bufs=3) as gsb, tc.tile_pool(
            name="logt", bufs=2
        ) as logtp, tc.tile_pool(name="gbig", bufs=2) as gbig, tc.tile_pool(
            name="psm", bufs=1, space="PSUM"
        ) as psm, tc.tile_pool(name="psg", bufs=1, space="PSUM") as psg:
            # per-batch persistent tiles
            maskb = [
                pbp.tile([P, ST], F32, tag=f"maskb{b}", name=f"maskb{b}")
                for b in range(B)
            ]
            cwb = [
                pbp.tile([P, ST], F32, tag=f"cwb{b}", name=f"cwb{b}")
                for b in range(B)
            ]
            idxt = [
                pbp.tile([P, SC], I32, tag=f"idxt{b}", name=f"idxt{b}")
                for b in range(B)
            ]
            idx16 = [
                pbp.tile([P, CAP // 16], I16, tag=f"idx16{b}", name=f"idx16{b}")
                for b in range(B)
            ]
            xgT = [
                pbp.tile([P, DC, CAP], BF16, tag=f"xgT{b}", name=f"xgT{b}")
                for b in range(B)
            ]
            omc = [
                pbp.tile([P, SC], F32, tag=f"omc{b}", name=f"omc{b}")
                for b in range(B)
            ]
            cspd = [
                pbp.tile([P, SC], F32, tag=f"cspd{b}", name=f"cspd{b}")
                for b in range(B)
            ]
            wv_w = [
                pbp.tile([P, SC], BF16, tag=f"wvw{b}", name=f"wvw{b}")
                for b in range(B)
            ]
            a_bf = [
                pbp.tile([1, CAP], BF16, tag=f"abf{b}", name=f"abf{b}")
                for b in range(B)
            ]

            def m_build(fcs):
                """M^T[d'-tile, d] = sum_f WkT[f, d']^T WqT[f, d] for d'-tiles
                in fcs. Each group accumulates over all 8 f chunks."""
                for dt in fcs:
                    for half in range(2):
                        pm = psm.tile(
                            [P, 512], F32, tag="m", bufs=2,
                            name=f"m{dt}_{half}",
                        )
                        for fc in range(DC):
                            nc.tensor.matmul(
                                pm[:],
                                wk_sb[:, fc, ts(dt, P)],
                                wq_sb[:, fc, ts(half, 512)],
                                start=(fc == 0),
                                stop=(fc == DC - 1),
                            )
                        nc.scalar.activation(
                            Msb[:, dt, ts(half, 512)], pm[:], AF.Copy
                        )

            def gate(b):
                xt = xtp.tile([P, DC, T], F32, tag="xt", name=f"xt{b}")
                nc.sync.dma_start(
                    xt[:],
                    xT.ap().rearrange("(c p) n -> p c n", p=P)[
                        :, :, b * T : (b + 1) * T
                    ],
                )
                logT = logtp.tile([8, T], F32, tag="logT", name=f"logT{b}")
                for half in range(2):
                    pg = psm.tile(
                        [P, 512], F32, tag="m", bufs=2, name=f"g{b}_{half}"
                    )
                    for dc in range(DC):
                        nc.tensor.matmul(
                            pg[0:E, :],
                            wg_sb[:, dc],
                            xt[:, dc, ts(half, 512)],
                            start=(dc == 0),
                            stop=(dc == DC - 1),
                        )
                    nc.scalar.activation(logT[:, ts(half, 512)], pg[0:E, :], AF.Copy)
                for tt in range(ST):
                    tp = psg.tile(
                        [P, 512], F32, tag="g", bufs=3, name=f"tp{b}_{tt}"
                    )
                    nc.tensor.transpose(
                        tp[:, 0:E], logT[:, ts(tt, P)], idn[0:E, 0:E]
                    )
                    gl = gsb.tile([P, E], F32, tag="gl")
                    nc.scalar.activation(gl[:], tp[:, 0:E], AF.Copy)
                    mx8 = gsb.tile([P, 8], F32, tag="mx8")
                    nc.vector.max(out=mx8[:], in_=gl[:])
                    mxn = gsb.tile([P, 1], F32, tag="mxn")
                    nc.vector.tensor_scalar_mul(mxn[:], mx8[:, 0:1], -1.0)
                    probs = gsb.tile([P, E], F32, tag="probs")
                    se = gsb.tile([P, 1], F32, tag="se")
                    nc.scalar.activation(
                        probs[:], gl[:], AF.Exp, bias=mxn[:], scale=1.0,
                        accum_out=se[:],
                    )
                    rs = gsb.tile([P, 1], F32, tag="rs")
                    nc.vector.reciprocal(rs[:], se[:])
                    nc.vector.tensor_scalar(
                        maskb[b][:, tt : tt + 1], gl[:, 0:1], mx8[:, 1:2], None,
                        op0=OP.is_ge,
                    )
                    nc.vector.scalar_tensor_tensor(
                        cwb[b][:, tt : tt + 1],
                        probs[:, 0:1],
                        rs[:],
                        maskb[b][:, tt : tt + 1],
                        op0=OP.mult,
                        op1=OP.mult,
                    )

            def gather(b):
                """Slot assignment by sparse_gather compaction, transposed
                bf16 x gather, omega weights.

                Build per-token values (global token id if routed else -1)
                in a 16-partition wrap, append CAP entries pointing at the
                zero x row, and let sparse_gather compact: output[j] = j-th
                routed token id for j < C, zero-row index NZ for j >= C."""
                t1 = gsb.tile([P, ST], F32, tag="t1")
                nc.vector.tensor_scalar(
                    t1[:], tvf[:], float(b * T + 1), None, op0=OP.add
                )
                nc.vector.tensor_mul(t1[:], t1[:], maskb[b][:])
                nc.vector.tensor_scalar(t1[:], t1[:], -1.0, None, op0=OP.add)
                nc.sync.dma_start(
                    mb_d[b].rearrange("(c p) -> p c", p=P), t1[:]
                )
                vals = gsb.tile([16, T // 16 + SC * 8], F32, tag="vals", bufs=2)
                nc.vector.memset(vals[:, T // 16 :], float(NZ))
                nc.sync.dma_start(
                    vals[:, 0 : T // 16],
                    mb_d[b].rearrange("(f p) -> p f", p=16),
                )
                idxw = gsb.tile([16, CAP // 16], F32, tag="idxw", bufs=2)
                nfu = gsb.tile([1, 1], mybir.dt.uint32, tag="nfu", bufs=2)
                nc.gpsimd.sparse_gather(idxw[:], vals[:], num_found=nfu[:])
                # C = num_found - CAP (the NZ pad entries are all counted)
                nfb = gsb.tile([P, 1], mybir.dt.uint32, tag="nfb")
                nc.gpsimd.partition_broadcast(nfb[:], nfu[:], channels=P)
                cf = gsb.tile([P, 1], F32, tag="cf")
                nc.vector.tensor_copy(cf[:], nfb[:])
                nc.vector.tensor_scalar(cf[:], cf[:], float(-CAP), None, op0=OP.add)
                # int16 lane-wrapped index list (replicated per 16-partition
                # group) for dma_gather
                nc.vector.tensor_copy(idx16[b][0:16, :], idxw[:])
                for r in range(1, P // 16):
                    nc.sync.dma_start(
                        idx16[b][16 * r : 16 * (r + 1), :], idx16[b][0:16, :]
                    )
                nc.gpsimd.dma_gather(
                    xgT[b][:],
                    xnb.ap(),
                    idx16[b][:],
                    num_idxs=CAP,
                    num_idxs_reg=CAP,
                    elem_size=D,
                    transpose=True,
                )
                # int32 slot->token in [P, SC] layout for the output scatter
                nc.sync.dma_start(
                    iw_d[b].rearrange("(f p) -> p f", p=16), idxw[:]
                )
                idxtf = gsb.tile([P, SC], F32, tag="idxtf")
                nc.sync.dma_start(
                    idxtf[:], iw_d[b].rearrange("(c p) -> p c", p=P)
                )
                nc.vector.tensor_copy(idxt[b][:], idxtf[:])
                # omega: 1 for j < C, (T - C) at j == CAP-1, else 0
                tmc = gsb.tile([P, 1], F32, tag="tmc")
                nc.vector.tensor_scalar(
                    tmc[:], cf[:], -1.0, float(T), op0=OP.mult, op1=OP.add
                )
                rep = gsb.tile([P, SC], F32, tag="rep")
                nc.vector.tensor_scalar(
                    omc[b][:], iosc[:], cf[:], None, op0=OP.is_lt
                )
                nc.vector.tensor_scalar(
                    rep[:], iosc[:], float(CAP - 1), None, op0=OP.is_equal
                )
                nc.vector.tensor_scalar(rep[:], rep[:], tmc[:], None, op0=OP.mult)
                nc.vector.tensor_add(omc[b][:], omc[b][:], rep[:])

                # --- a, c, vw vectors: [3, CAP] = w3^T x_g^T (+ biases) ---
                p3 = psg.tile([P, 512], F32, tag="g", bufs=3, name=f"p3{b}")
                for dc in range(DC):
                    nc.tensor.matmul(
                        p3[0:3, 0:CAP],
                        w3_sb[:, dc],
                        xgT[b][:, dc],
                        start=(dc == 0),
                        stop=(dc == DC - 1),
                    )
                avc = gsb.tile([3, CAP], F32, tag="avc")
                nc.scalar.activation(
                    avc[:], p3[0:3, 0:CAP], AF.Identity, bias=bias3[:]
                )
                nc.vector.tensor_copy(a_bf[b][:], avc[0:1, :])
                nc.sync.dma_start(cu_d[b][:], avc[1:3, :])
                nc.sync.dma_start(
                    cspd[b][:], cu_d[b][0].rearrange("(c p) -> p c", p=P)
                )
                vwsp = gsb.tile([P, SC], F32, tag="vwsp")
                nc.sync.dma_start(
                    vwsp[:], cu_d[b][1].rearrange("(c p) -> p c", p=P)
                )
                nc.vector.tensor_scalar_mul(cspd[b][:], cspd[b][:], 1.0 / D)
                vww = gsb.tile([P, SC], F32, tag="vww")
                nc.vector.tensor_mul(vww[:], vwsp[:], omc[b][:])
                nc.vector.tensor_copy(wv_w[b][:], vww[:])

            def block(b):
                """Per-batch dense compute: G' = x_g M^T, S, exp, Z, num,
                combine."""
                # --- G'^T[d, s] = sum_d' M^T[d', d] xgT[d', s] ---
                Gsb = gbig.tile([P, DC, CAP], BF16, tag="Gsb", name=f"Gsb{b}")
                for dt in range(DC):
                    pgt = psg.tile(
                        [P, 512], F32, tag="g", bufs=3, name=f"G{b}_{dt}"
                    )
                    for dc in range(DC):
                        nc.tensor.matmul(
                            pgt[:, 0:CAP],
                            Msb[:, dc, ts(dt, P)],
                            xgT[b][:, dc],
                            start=(dc == 0),
                            stop=(dc == DC - 1),
                        )
                    nc.scalar.activation(Gsb[:, dt], pgt[:, 0:CAP], AF.Copy)

                # --- S[s, t] = sum_d G'^T[d, s] xgT[d, t] + a[t] (+c[s]/D
                # via exp bias); E = exp(S/D) with free-axis row sums ---
                Eg = gbig.tile([P, SC, CAP], BF16, tag="Eg", name=f"Eg{b}")
                eacc = gsb.tile([P, SC], F32, tag="eacc")
                erw = gsb.tile([P, SC], F32, tag="erw")
                for st in range(SC):
                    pss = psg.tile(
                        [P, 512], F32, tag="g", bufs=3, name=f"S{b}_{st}"
                    )
                    for dt in range(DC):
                        nc.tensor.matmul(
                            pss[:, 0:CAP],
                            Gsb[:, dt, ts(st, P)],
                            xgT[b][:, dt],
                            start=(dt == 0),
                            stop=False,
                        )
                    nc.tensor.matmul(
                        pss[:, 0:CAP],
                        ones_bf[:],
                        a_bf[b][:],
                        start=False,
                        stop=True,
                    )
                    nc.scalar.activation(
                        Eg[:, st], pss[:, 0:CAP], AF.Exp,
                        bias=cspd[b][:, st : st + 1], scale=float(1.0 / D),
                        accum_out=eacc[:, st : st + 1],
                    )
                    nc.vector.tensor_scalar(
                        erw[:, st : st + 1],
                        Eg[:, st, CAP - 1 : CAP],
                        float(T - CAP),
                        eacc[:, st : st + 1],
                        op0=OP.mult,
                        op1=OP.add,
                    )
                # Z = omega . erw (both [slot] in partition-chunk layout)
                scr3 = gsb.tile([P, SC], F32, tag="scr3")
                zp = gsb.tile([P, 1], F32, tag="zp")
                nc.vector.tensor_mul(scr3[:], erw[:], omc[b][:])
                nc.vector.reduce_sum(zp[:], scr3[:], axis=AX.X)
                za = gsb.tile([P, 1], F32, tag="za")
                nc.gpsimd.partition_all_reduce(
                    za[:], zp[:], channels=P, reduce_op=RED.add
                )
                rZ = gsb.tile([P, 1], F32, tag="rZ")
                nc.vector.reciprocal(rZ[:], za[:])
                # num[t] = sum_s E[s, t] (omega_s vw[s])
                numg = gsb.tile([P, SC], F32, tag="numg")
                for ti in range(SC):
                    pn = psg.tile(
                        [P, 512], F32, tag="g", bufs=3, name=f"pn{b}_{ti}"
                    )
                    for sc in range(SC):
                        nc.tensor.matmul(
                            pn[:, 0:1],
                            Eg[:, sc, ts(ti, P)],
                            wv_w[b][:, sc : sc + 1],
                            start=(sc == 0),
                            stop=(sc == SC - 1),
                        )
                    nc.scalar.activation(numg[:, ti : ti + 1], pn[:, 0:1], AF.Copy)
                # out_sum = num/Z + boS; scatter to token space
                outg = gsb.tile([P, SC], F32, tag="outg")
                nc.vector.tensor_scalar(
                    outg[:], numg[:], rZ[:], boS_bc[:], op0=OP.mult, op1=OP.add
                )
                for i in range(SC):
                    nc.gpsimd.indirect_dma_start(
                        out=sc_d[:, None],
                        out_offset=bass.IndirectOffsetOnAxis(
                            ap=idxt[b][:, i : i + 1], axis=0
                        ),
                        in_=outg[:, i : i + 1],
                        in_offset=None,
                        bounds_check=N - 1,
                        oob_is_err=False,
                    )
                scb = gsb.tile([P, ST], F32, tag="scb")
                nc.sync.dma_start(
                    scb[:],
                    sc_d[b * T : (b + 1) * T].rearrange("(c p) -> p c", p=P),
                )
                ob = gsb.tile([P, ST], F32, tag="ob")
                nc.vector.tensor_mul(ob[:], scb[:], cwb[b][:])
                nc.sync.dma_start(out_d.ap()[b], ob[:])

            # ---------------- pipeline ----------------
            wq_sb = wqk.tile([P, DC, D], BF16, tag="wq", name="wq")
            wk_sb = wqk.tile([P, DC, D], BF16, tag="wk", name="wk")
            nc.sync.dma_start(
                wq_sb[:], wqT_d.ap().rearrange("(c p) d -> p c d", p=P)
            )
            nc.sync.dma_start(
                wk_sb[:], wkT_d.ap().rearrange("(c p) d -> p c d", p=P)
            )
            # zero the token-space scatter target once
            for bb in range(B):
                nc.sync.dma_start(
                    sc_d[bb * T : (bb + 1) * T].rearrange("(c p) -> p c", p=P),
                    zt[:],
                )
            m_build(range(DC))
            gate(0)
            gather(0)
            gate(1)
            gather(1)
            gate(2)
            gather(2)
            gate(3)
            gather(3)
            block(0)
            block(1)
            block(2)
            block(3)


def build_nc():
    nc = bacc.Bacc("TRN2", target_bir_lowering=False, debug=False, num_devices=8)
    xT = nc.dram_tensor("xT", [D, N], F32, kind="ExternalInput")
    xnb = nc.dram_tensor("xnb", [N + 1, D], BF16, kind="ExternalInput")
    wg_d = nc.dram_tensor("wg", [D, E], F32, kind="ExternalInput")
    wqT_d = nc.dram_tensor("wqT", [DH, D], BF16, kind="ExternalInput")
    wkT_d = nc.dram_tensor("wkT", [DH, D], BF16, kind="ExternalInput")
    w3_d = nc.dram_tensor("w3", [D, 3], BF16, kind="ExternalInput")
    sconst_d = nc.dram_tensor("sconst", [8], F32, kind="ExternalInput")
    out_d = nc.dram_tensor("contrib", [B, P, ST], F32, kind="ExternalOutput")
    with tile.TileContext(nc) as tc:
        _emit(
            nc,
            tc,
            (xT, xnb, wg_d, wqT_d, wkT_d, w3_d, sconst_d),
            (out_d,),
        )
    nc.compile()
    return nc


def make_in_maps(x, wg, wqkv, bqkv, wo, bo):
    xn = np.ascontiguousarray(x.reshape(N, D), dtype=np.float32)
    xT = np.ascontiguousarray(xn.T)
    xnb = np.zeros((N + 1, D), dtype=ml_dtypes.bfloat16)
    xnb[:N] = xn
    in_maps = []
    for e in range(E):
        perm = [e] + [j for j in range(E) if j != e]
        wq = wqkv[e][:, 0::3].astype(np.float64)
        wk = wqkv[e][:, 1::3].astype(np.float64)
        wv = wqkv[e][:, 2::3].astype(np.float64)
        bq = bqkv[e][0::3].astype(np.float64)
        bk = bqkv[e][1::3].astype(np.float64)
        bv = bqkv[e][2::3].astype(np.float64)
        wos = wo[e].astype(np.float64).sum(axis=1)
        u = wv @ wos
        aw = wq @ bk
        cv = wk @ bq
        cc = float(bq @ bk)
        c0 = float(bv @ wos)
        boS = float(bo[e].astype(np.float64).sum())
        in_maps.append(
            {
                "xT": xT,
                "xnb": xnb,
                "wg": np.ascontiguousarray(wg[:, perm], dtype=np.float32),
                "wqT": np.ascontiguousarray(wq.T).astype(ml_dtypes.bfloat16),
                "wkT": np.ascontiguousarray(wk.T).astype(ml_dtypes.bfloat16),
                "w3": np.ascontiguousarray(
                    np.stack([aw, cv, u], axis=1)
                ).astype(ml_dtypes.bfloat16),
                "sconst": np.array(
                    [cc, 0.0, c0, boS, 0.0, 0.0, 0.0, 0.0], dtype=np.float32
                ),
            }
        )
    return in_maps


def run_device(in_maps, trace=False):
    if "nc" not in _CACHE:
        _CACHE["nc"] = build_nc()
    return bass_utils.run_bass_kernel_spmd(
        _CACHE["nc"], in_maps, core_ids=list(range(E)), trace=trace
    )


def kernel(x, wg, wqkv, bqkv, wo, bo, top_k):
    assert int(top_k) == 2, f"kernel hardcodes top_k=2, got {top_k}"
    x = np.asarray(x, np.float32)
    wg = np.asarray(wg, np.float32)
    wqkv = np.asarray(wqkv, np.float32)
    bqkv = np.asarray(bqkv, np.float32)
    wo = np.asarray(wo, np.float32)
    bo = np.asarray(bo, np.float32)

    res = run_device(make_in_maps(x, wg, wqkv, bqkv, wo, bo))
    total = np.zeros((B, T), np.float64)
    for c in range(E):
        contrib = res.results[c]["contrib"]  # [B, P, ST], t = tt*128 + p
        total += contrib.transpose(0, 2, 1).reshape(B, T).astype(np.float64)
    m = total.max(axis=1, keepdims=True)
    ls = total - m - np.log(np.exp(total - m).sum(axis=1, keepdims=True))
    return ls.astype(np.float32)


# revision 25
# speedup vs baseline: 1.3652x; 1.1173x over previous
"""MoE-routing attention kernel for 8 Trainium2 NeuronCores — v2.

Expert parallelism (1 expert per core), full inputs in, full output out.
v2 restructures v1 around three ideas:

1. Merged q/k projection: S[s,t] = q_t.k_s = x_t M x_s + a[t] + c[s] + cc
   with M = Wq Wk^T precomputed ON DEVICE once per expert (dense 1024^3
   matmul that also warms the PE), a = x.(Wq bk), c = x.(Wk bq),
   cc = bq.bk. Per batch this is ONE projection G' = x_g M^T (64 MMs)
   plus 24 score MMs instead of two projections (128 MMs) + scores.
2. bf16 everywhere on the S path (validated: rel err stays at 1e-6
   because scores are divided by D=1024 before exp). bf16 halves
   LDWEIGHTS cost (FWL) and DMA volume.
3. Gathered x arrives TRANSPOSED directly via dma_gather(transpose=True)
   (bf16, one instruction per batch) — no PE transposes, no PSUM copies.
   Pad slots point at a host-appended zero row of x.

The gate runs with wg STATIONARY (8 LDWEIGHTS total instead of 256):
logits^T = wg^T @ x^T in [8, N] layout, then 8 tiny PE transposes per
batch back to [token, 8] for the top-2/softmax tail (fp32 - routing
flips are the one numerically fragile spot).

Weight-derived vectors (u = wv @ wo_rowsum, aw = Wq bk, cv = Wk bq and
scalars cc, c0 = bv.wo_rowsum, boS = sum bo) are folded on the host like
bias preprocessing. All model FLOPs (gate, M, G', S, attention combine)
stay on device.

Host: sums the 8 per-core [B,T] contribution vectors and applies the
final log_softmax (the SPMD combine/unshard step), as in v1.
"""

import sys

import numpy as np

for _p in ("/opt/trn_rl_repo", "/root/.axon_site/_ro/trn_rl_repo"):
    if _p not in sys.path:
        sys.path.append(_p)

import ml_dtypes  # noqa: E402

import concourse.bass as bass  # noqa: E402
import concourse.bass_isa as bass_isa  # noqa: E402
import concourse.mybir as mybir  # noqa: E402
import concourse.tile as tile  # noqa: E402
from concourse import bacc  # noqa: E402
from concourse import bass_utils  # noqa: E402
from concourse.bass import ts  # noqa: E402

P = 128
B, T, D, E = 4, 1024, 1024, 8
DH = D
N = B * T
DC = D // P  # 8 contraction chunks
ST = T // P  # 8 token tiles per batch
CAP = 384  # gathered slot capacity per (expert, batch)
SC = CAP // P  # 3 slot tiles
NZ = N  # index of the host-appended all-zero x row (pad target)
BIG = 1 << 20
F32 = mybir.dt.float32
F32R = mybir.dt.float32r
BF16 = mybir.dt.bfloat16
I32 = mybir.dt.int32
I16 = mybir.dt.int16
AF = mybir.ActivationFunctionType
OP = mybir.AluOpType
AX = mybir.AxisListType
RED = bass_isa.ReduceOp

_CACHE = {}


TS = N // E  # tokens per core's gate shard


def _emit(nc, tc, dt_in, dt_out):
    (xTs, xnb, wg_d, wqT_d, wkT_d, w3_d, sconst_d) = dt_in
    (out_d,) = dt_out

    with tc.tile_pool(name="const", bufs=1) as const, tc.tile_pool(
        name="wqk", bufs=1
    ) as wqk, tc.tile_pool(name="msb", bufs=1) as msbp, tc.tile_pool(
        name="drams", bufs=1, space="DRAM"
    ) as dramp:
        # ---------------- constants ----------------
        wg_sb = const.tile([P, DC, E], F32)
        nc.sync.dma_start(wg_sb[:], wg_d.ap().rearrange("(c p) e -> p c e", p=P))
        w3_sb = const.tile([P, DC, 3], BF16)
        nc.sync.dma_start(w3_sb[:], w3_d.ap().rearrange("(c p) k -> p c k", p=P))
        bias3 = const.tile([3, 1], F32)
        nc.sync.dma_start(bias3[:], sconst_d.ap()[0:3, None])
        boS_bc = const.tile([P, 1], F32)
        nc.sync.dma_start(
            boS_bc[:], sconst_d.ap()[3:4][None, :].to_broadcast([P, 1])
        )

        ones_bf = const.tile([1, P], BF16)
        nc.vector.memset(ones_bf[:], 1.0)
        iosc_i = const.tile([P, SC], I32)  # value = slot j = c*128 + p
        nc.gpsimd.iota(iosc_i[:], pattern=[[P, SC]], base=0, channel_multiplier=1)
        iosc = const.tile([P, SC], F32)
        nc.vector.tensor_copy(iosc[:], iosc_i[:])
        tk16_i = const.tile([16, T // 16], I32)  # token id t = f*16 + p
        nc.gpsimd.iota(
            tk16_i[:], pattern=[[16, T // 16]], base=0, channel_multiplier=1
        )
        tk16 = const.tile([16, T // 16], F32)
        nc.vector.tensor_copy(tk16[:], tk16_i[:])
        zt = const.tile([P, ST], F32)
        nc.vector.memset(zt[:], 0.0)

        Msb = msbp.tile([P, DC, D], BF16)  # M^T[d', d]; chunk dc = d' rows

        sc_d = dramp.tile([N], F32, tag="scd", name="scd")
        a2a_in = dramp.tile([E, 2, TS], F32, tag="a2ain", name="a2ain")
        a2a_out = dramp.tile([E, 2, TS], F32, tag="a2aout", name="a2aout")
        iw_d = [
            dramp.tile([CAP], F32, tag=f"iwd{b}", name=f"iwd{b}")
            for b in range(B)
        ]
        cu_d = [
            dramp.tile([2, CAP], F32, tag=f"cud{b}", name=f"cud{b}")
            for b in range(B)
        ]

        with tc.tile_pool(name="pb", bufs=1) as pbp, tc.tile_pool(
            name="gsb", bufs=3
        ) as gsb, tc.tile_pool(name="gbig", bufs=2) as gbig, tc.tile_pool(
            name="psm", bufs=1, space="PSUM"
        ) as psm, tc.tile_pool(name="psg", bufs=1, space="PSUM") as psg:
            # per-batch persistent tiles
            cwb = [
                pbp.tile([P, ST], F32, tag=f"cwb{b}", name=f"cwb{b}")
                for b in range(B)
            ]
            idxt = [
                pbp.tile([P, SC], I32, tag=f"idxt{b}", name=f"idxt{b}")
                for b in range(B)
            ]
            idx16 = [
                pbp.tile([P, CAP // 16], I16, tag=f"idx16{b}", name=f"idx16{b}")
                for b in range(B)
            ]
            xgT = [
                pbp.tile([P, DC, CAP], BF16, tag=f"xgT{b}", name=f"xgT{b}")
                for b in range(B)
            ]
            omc = [
                pbp.tile([P, SC], F32, tag=f"omc{b}", name=f"omc{b}")
                for b in range(B)
            ]
            cspd = [
                pbp.tile([P, SC], F32, tag=f"cspd{b}", name=f"cspd{b}")
                for b in range(B)
            ]
            wv_w = [
                pbp.tile([P, SC], BF16, tag=f"wvw{b}", name=f"wvw{b}")
                for b in range(B)
            ]
            a_bf = [
                pbp.tile([1, CAP], BF16, tag=f"abf{b}", name=f"abf{b}")
                for b in range(B)
            ]

            def m_build(fcs):
                """M^T[d'-tile, d] = sum_f WkT[f, d']^T WqT[f, d] for d'-tiles
                in fcs. Each group accumulates over all 8 f chunks."""
                for dt in fcs:
                    for half in range(2):
                        pm = psm.tile(
                            [P, 512], F32, tag="m", bufs=2,
                            name=f"m{dt}_{half}",
                        )
                        for fc in range(DC):
                            nc.tensor.matmul(
                                pm[:],
                                wk_sb[:, fc, ts(dt, P)],
                                wq_sb[:, fc, ts(half, 512)],
                                start=(fc == 0),
                                stop=(fc == DC - 1),
                            )
                        nc.scalar.activation(
                            Msb[:, dt, ts(half, 512)], pm[:], AF.Copy
                        )

            def gate():
                """Sharded gate: this core computes logits for its own
                TS-token slice (full fp32), top-2 + softmax for ALL experts
                in [8, TS] layout, then AllToAll so every core ends up with
                its own expert's mask/cw over all N tokens."""
                xts = pbp.tile([P, DC, TS], F32, tag="xts", name="xts")
                nc.sync.dma_start(
                    xts[:], xTs.ap().rearrange("(c p) n -> p c n", p=P)
                )
                pg = psm.tile([P, 512], F32, tag="m", bufs=2, name="pgate")
                for dc in range(DC):
                    nc.tensor.matmul(
                        pg[0:E, 0:TS],
                        wg_sb[:, dc],
                        xts[:, dc],
                        start=(dc == 0),
                        stop=(dc == DC - 1),
                    )
                lsh = gsb.tile([E, TS], F32, tag="lsh")
                nc.scalar.activation(lsh[:], pg[0:E, 0:TS], AF.Copy)
                m1 = gsb.tile([E, TS], F32, tag="m1")
                nc.gpsimd.partition_all_reduce(
                    m1[:], lsh[:], channels=E, reduce_op=RED.max
                )
                lm = gsb.tile([E, TS], F32, tag="lm")
                nc.vector.tensor_tensor(lm[:], lsh[:], m1[:], op=OP.is_ge)
                nc.vector.scalar_tensor_tensor(
                    lm[:], lm[:], float(-BIG), lsh[:], op0=OP.mult, op1=OP.add
                )
                m2 = gsb.tile([E, TS], F32, tag="m2")
                nc.gpsimd.partition_all_reduce(
                    m2[:], lm[:], channels=E, reduce_op=RED.max
                )
                pk = gsb.tile([E, TS], F32, tag="pk")
                nc.vector.tensor_tensor(pk[:], lsh[:], m2[:], op=OP.is_ge)
                ex = gsb.tile([E, TS], F32, tag="ex")
                nc.vector.tensor_tensor(ex[:], lsh[:], m1[:], op=OP.subtract)
                nc.scalar.activation(ex[:], ex[:], AF.Exp)
                se = gsb.tile([E, TS], F32, tag="se")
                nc.gpsimd.partition_all_reduce(
                    se[:], ex[:], channels=E, reduce_op=RED.add
                )
                rse = gsb.tile([E, TS], F32, tag="rse")
                nc.vector.reciprocal(rse[:], se[:])
                cw8 = gsb.tile([E, TS], F32, tag="cw8")
                nc.vector.tensor_mul(ex[:], ex[:], pk[:])
                nc.vector.tensor_mul(cw8[:], ex[:], rse[:])
                nc.sync.dma_start(a2a_in[:, 0, :], pk[:])
                nc.sync.dma_start(a2a_in[:, 1, :], cw8[:])
                nc.gpsimd.collective_compute(
                    "AllToAll",
                    mybir.AluOpType.bypass,
                    replica_groups=[list(range(E))],
                    ins=[a2a_in[:]],
                    outs=[a2a_out[:]],
                )
                for b in range(B):
                    for r in range(2):
                        nc.sync.dma_start(
                            cwb[b][:, 4 * r : 4 * (r + 1)],
                            a2a_out[2 * b + r, 1, :].rearrange(
                                "(c p) -> p c", p=P
                            ),
                        )

            def gather(b):
                """Slot assignment by sparse_gather compaction, transposed
                bf16 x gather, omega weights.

                Build per-token values (global token id if routed else -1)
                in a 16-partition wrap, append CAP entries pointing at the
                zero x row, and let sparse_gather compact: output[j] = j-th
                routed token id for j < C, zero-row index NZ for j >= C."""
                vals = gsb.tile([16, T // 16 + SC * 8], F32, tag="vals", bufs=2)
                nc.vector.memset(vals[:, T // 16 :], float(NZ))
                for r in range(2):
                    nc.sync.dma_start(
                        vals[:, 32 * r : 32 * (r + 1)],
                        a2a_out[2 * b + r, 0, :].rearrange(
                            "(f p) -> p f", p=16
                        ),
                    )
                # mask -> global token id if routed else -1
                t1 = gsb.tile([16, T // 16], F32, tag="t1")
                nc.vector.tensor_scalar(
                    t1[:], tk16[:], float(b * T + 1), None, op0=OP.add
                )
                nc.vector.tensor_mul(t1[:], t1[:], vals[:, 0 : T // 16])
                nc.vector.tensor_scalar(
                    vals[:, 0 : T // 16], t1[:], -1.0, None, op0=OP.add
                )
                idxw = gsb.tile([16, CAP // 16], F32, tag="idxw", bufs=2)
                nfu = gsb.tile([1, 1], mybir.dt.uint32, tag="nfu", bufs=2)
                nc.gpsimd.sparse_gather(idxw[:], vals[:], num_found=nfu[:])
                # C = num_found - CAP (the NZ pad entries are all counted)
                nfb = gsb.tile([P, 1], mybir.dt.uint32, tag="nfb")
                nc.gpsimd.partition_broadcast(nfb[:], nfu[:], channels=P)
                cf = gsb.tile([P, 1], F32, tag="cf")
                nc.vector.tensor_copy(cf[:], nfb[:])
                nc.vector.tensor_scalar(cf[:], cf[:], float(-CAP), None, op0=OP.add)
                # int16 lane-wrapped index list (replicated per 16-partition
                # group) for dma_gather
                nc.vector.tensor_copy(idx16[b][0:16, :], idxw[:])
                for r in range(1, P // 16):
                    nc.sync.dma_start(
                        idx16[b][16 * r : 16 * (r + 1), :], idx16[b][0:16, :]
                    )
                nc.gpsimd.dma_gather(
                    xgT[b][:],
                    xnb.ap(),
                    idx16[b][:],
                    num_idxs=CAP,
                    num_idxs_reg=CAP,
                    elem_size=D,
                    transpose=True,
                )
                # int32 slot->token in [P, SC] layout for the output scatter
                nc.sync.dma_start(
                    iw_d[b].rearrange("(f p) -> p f", p=16), idxw[:]
                )
                idxtf = gsb.tile([P, SC], F32, tag="idxtf")
                nc.sync.dma_start(
                    idxtf[:], iw_d[b].rearrange("(c p) -> p c", p=P)
                )
                nc.vector.tensor_copy(idxt[b][:], idxtf[:])
                # omega: 1 for j < C, (T - C) at j == CAP-1, else 0
                tmc = gsb.tile([P, 1], F32, tag="tmc")
                nc.vector.tensor_scalar(
                    tmc[:], cf[:], -1.0, float(T), op0=OP.mult, op1=OP.add
                )
                rep = gsb.tile([P, SC], F32, tag="rep")
                nc.vector.tensor_scalar(
                    omc[b][:], iosc[:], cf[:], None, op0=OP.is_lt
                )
                nc.vector.tensor_scalar(
                    rep[:], iosc[:], float(CAP - 1), None, op0=OP.is_equal
                )
                nc.vector.tensor_scalar(rep[:], rep[:], tmc[:], None, op0=OP.mult)
                nc.vector.tensor_add(omc[b][:], omc[b][:], rep[:])

                # --- a, c, vw vectors: [3, CAP] = w3^T x_g^T (+ biases) ---
                p3 = psg.tile([P, 512], F32, tag="g", bufs=3, name=f"p3{b}")
                for dc in range(DC):
                    nc.tensor.matmul(
                        p3[0:3, 0:CAP],
                        w3_sb[:, dc],
                        xgT[b][:, dc],
                        start=(dc == 0),
                        stop=(dc == DC - 1),
                    )
                avc = gsb.tile([3, CAP], F32, tag="avc")
                nc.scalar.activation(
                    avc[:], p3[0:3, 0:CAP], AF.Identity, bias=bias3[:]
                )
                nc.vector.tensor_copy(a_bf[b][:], avc[0:1, :])
                nc.sync.dma_start(cu_d[b][:], avc[1:3, :])
                nc.sync.dma_start(
                    cspd[b][:], cu_d[b][0].rearrange("(c p) -> p c", p=P)
                )
                vwsp = gsb.tile([P, SC], F32, tag="vwsp")
                nc.sync.dma_start(
                    vwsp[:], cu_d[b][1].rearrange("(c p) -> p c", p=P)
                )
                nc.vector.tensor_scalar_mul(cspd[b][:], cspd[b][:], 1.0 / D)
                vww = gsb.tile([P, SC], F32, tag="vww")
                nc.vector.tensor_mul(vww[:], vwsp[:], omc[b][:])
                nc.vector.tensor_copy(wv_w[b][:], vww[:])

            def block(b):
                """Per-batch dense compute: G' = x_g M^T, S, exp, Z, num,
                combine."""
                # --- G'^T[d, s] = sum_d' M^T[d', d] xgT[d', s] ---
                Gsb = gbig.tile([P, DC, CAP], BF16, tag="Gsb", name=f"Gsb{b}")
                for dt in range(DC):
                    pgt = psg.tile(
                        [P, 512], F32, tag="g", bufs=3, name=f"G{b}_{dt}"
                    )
                    for dc in range(DC):
                        nc.tensor.matmul(
                            pgt[:, 0:CAP],
                            Msb[:, dc, ts(dt, P)],
                            xgT[b][:, dc],
                            start=(dc == 0),
                            stop=(dc == DC - 1),
                        )
                    nc.scalar.activation(Gsb[:, dt], pgt[:, 0:CAP], AF.Copy)

                # --- S[s, t] = sum_d G'^T[d, s] xgT[d, t] + a[t] (+c[s]/D
                # via exp bias); E = exp(S/D) with free-axis row sums ---
                Eg = gbig.tile([P, SC, CAP], BF16, tag="Eg", name=f"Eg{b}")
                eacc = gsb.tile([P, SC], F32, tag="eacc")
                erw = gsb.tile([P, SC], F32, tag="erw")
                for st in range(SC):
                    pss = psg.tile(
                        [P, 512], F32, tag="g", bufs=3, name=f"S{b}_{st}"
                    )
                    for dt in range(DC):
                        nc.tensor.matmul(
                            pss[:, 0:CAP],
                            Gsb[:, dt, ts(st, P)],
                            xgT[b][:, dt],
                            start=(dt == 0),
                            stop=False,
                        )
                    nc.tensor.matmul(
                        pss[:, 0:CAP],
                        ones_bf[:],
                        a_bf[b][:],
                        start=False,
                        stop=True,
                    )
                    nc.scalar.activation(
                        Eg[:, st], pss[:, 0:CAP], AF.Exp,
                        bias=cspd[b][:, st : st + 1], scale=float(1.0 / D),
                        accum_out=eacc[:, st : st + 1],
                    )
                    nc.vector.tensor_scalar(
                        erw[:, st : st + 1],
                        Eg[:, st, CAP - 1 : CAP],
                        float(T - CAP),
                        eacc[:, st : st + 1],
                        op0=OP.mult,
                        op1=OP.add,
                    )
                # Z = omega . erw (both [slot] in partition-chunk layout)
                scr3 = gsb.tile([P, SC], F32, tag="scr3")
                zp = gsb.tile([P, 1], F32, tag="zp")
                nc.vector.tensor_mul(scr3[:], erw[:], omc[b][:])
                nc.vector.reduce_sum(zp[:], scr3[:], axis=AX.X)
                za = gsb.tile([P, 1], F32, tag="za")
                nc.gpsimd.partition_all_reduce(
                    za[:], zp[:], channels=P, reduce_op=RED.add
                )
                rZ = gsb.tile([P, 1], F32, tag="rZ")
                nc.vector.reciprocal(rZ[:], za[:])
                # num[t] = sum_s E[s, t] (omega_s vw[s])
                numg = gsb.tile([P, SC], F32, tag="numg")
                for ti in range(SC):
                    pn = psg.tile(
                        [P, 512], F32, tag="g", bufs=3, name=f"pn{b}_{ti}"
                    )
                    for sc in range(SC):
                        nc.tensor.matmul(
                            pn[:, 0:1],
                            Eg[:, sc, ts(ti, P)],
                            wv_w[b][:, sc : sc + 1],
                            start=(sc == 0),
                            stop=(sc == SC - 1),
                        )
                    nc.scalar.activation(numg[:, ti : ti + 1], pn[:, 0:1], AF.Copy)
                # out_sum = num/Z + boS; scatter to token space
                outg = gsb.tile([P, SC], F32, tag="outg")
                nc.vector.tensor_scalar(
                    outg[:], numg[:], rZ[:], boS_bc[:], op0=OP.mult, op1=OP.add
                )
                for i in range(SC):
                    nc.gpsimd.indirect_dma_start(
                        out=sc_d[:, None],
                        out_offset=bass.IndirectOffsetOnAxis(
                            ap=idxt[b][:, i : i + 1], axis=0
                        ),
                        in_=outg[:, i : i + 1],
                        in_offset=None,
                        bounds_check=N - 1,
                        oob_is_err=False,
                    )
                scb = gsb.tile([P, ST], F32, tag="scb")
                nc.sync.dma_start(
                    scb[:],
                    sc_d[b * T : (b + 1) * T].rearrange("(c p) -> p c", p=P),
                )
                ob = gsb.tile([P, ST], F32, tag="ob")
                nc.vector.tensor_mul(ob[:], scb[:], cwb[b][:])
                nc.sync.dma_start(out_d.ap()[b], ob[:])

            # ---------------- pipeline ----------------
            wq_sb = wqk.tile([P, DC, D], BF16, tag="wq", name="wq")
            wk_sb = wqk.tile([P, DC, D], BF16, tag="wk", name="wk")
            nc.sync.dma_start(
                wq_sb[:], wqT_d.ap().rearrange("(c p) d -> p c d", p=P)
            )
            nc.sync.dma_start(
                wk_sb[:], wkT_d.ap().rearrange("(c p) d -> p c d", p=P)
            )
            # zero the token-space scatter target once
            for bb in range(B):
                nc.sync.dma_start(
                    sc_d[bb * T : (bb + 1) * T].rearrange("(c p) -> p c", p=P),
                    zt[:],
                )
            gate()
            m_build(range(DC))
            gather(0)
            gather(1)
            gather(2)
            gather(3)
            block(0)
            block(1)
            block(2)
            block(3)


def build_nc():
    nc = bacc.Bacc("TRN2", target_bir_lowering=False, debug=False, num_devices=8)
    xTs = nc.dram_tensor("xTs", [D, TS], F32, kind="ExternalInput")
    xnb = nc.dram_tensor("xnb", [N + 1, D], BF16, kind="ExternalInput")
    wg_d = nc.dram_tensor("wg", [D, E], F32, kind="ExternalInput")
    wqT_d = nc.dram_tensor("wqT", [DH, D], BF16, kind="ExternalInput")
    wkT_d = nc.dram_tensor("wkT", [DH, D], BF16, kind="ExternalInput")
    w3_d = nc.dram_tensor("w3", [D, 3], BF16, kind="ExternalInput")
    sconst_d = nc.dram_tensor("sconst", [8], F32, kind="ExternalInput")
    out_d = nc.dram_tensor("contrib", [B, P, ST], F32, kind="ExternalOutput")
    with tile.TileContext(nc) as tc:
        _emit(
            nc,
            tc,
            (xTs, xnb, wg_d, wqT_d, wkT_d, w3_d, sconst_d),
            (out_d,),
        )
    nc.compile()
    return nc


def make_in_maps(x, wg, wqkv, bqkv, wo, bo):
    xn = np.ascontiguousarray(x.reshape(N, D), dtype=np.float32)
    xT = np.ascontiguousarray(xn.T)
    wg32 = np.ascontiguousarray(wg, dtype=np.float32)
    xnb = np.zeros((N + 1, D), dtype=ml_dtypes.bfloat16)
    xnb[:N] = xn
    in_maps = []
    for e in range(E):
        wq = wqkv[e][:, 0::3].astype(np.float64)
        wk = wqkv[e][:, 1::3].astype(np.float64)
        wv = wqkv[e][:, 2::3].astype(np.float64)
        bq = bqkv[e][0::3].astype(np.float64)
        bk = bqkv[e][1::3].astype(np.float64)
        bv = bqkv[e][2::3].astype(np.float64)
        wos = wo[e].astype(np.float64).sum(axis=1)
        u = wv @ wos
        aw = wq @ bk
        cv = wk @ bq
        cc = float(bq @ bk)
        c0 = float(bv @ wos)
        boS = float(bo[e].astype(np.float64).sum())
        in_maps.append(
            {
                "xTs": np.ascontiguousarray(xT[:, e * TS : (e + 1) * TS]),
                "xnb": xnb,
                "wg": wg32,
                "wqT": np.ascontiguousarray(wq.T).astype(ml_dtypes.bfloat16),
                "wkT": np.ascontiguousarray(wk.T).astype(ml_dtypes.bfloat16),
                "w3": np.ascontiguousarray(
                    np.stack([aw, cv, u], axis=1)
                ).astype(ml_dtypes.bfloat16),
                "sconst": np.array(
                    [cc, 0.0, c0, boS, 0.0, 0.0, 0.0, 0.0], dtype=np.float32
                ),
            }
        )
    return in_maps


def run_device(in_maps, trace=False):
    if "nc" not in _CACHE:
        _CACHE["nc"] = build_nc()
    return bass_utils.run_bass_kernel_spmd(
        _CACHE["nc"], in_maps, core_ids=list(range(E)), trace=trace
    )


def kernel(x, wg, wqkv, bqkv, wo, bo, top_k):
    assert int(top_k) == 2, f"kernel hardcodes top_k=2, got {top_k}"
    x = np.asarray(x, np.float32)
    wg = np.asarray(wg, np.float32)
    wqkv = np.asarray(wqkv, np.float32)
    bqkv = np.asarray(bqkv, np.float32)
    wo = np.asarray(wo, np.float32)
    bo = np.asarray(bo, np.float32)

    res = run_device(make_in_maps(x, wg, wqkv, bqkv, wo, bo))
    total = np.zeros((B, T), np.float64)
    for c in range(E):
        contrib = res.results[c]["contrib"]  # [B, P, ST], t = tt*128 + p
        total += contrib.transpose(0, 2, 1).reshape(B, T).astype(np.float64)
    m = total.max(axis=1, keepdims=True)
    ls = total - m - np.log(np.exp(total - m).sum(axis=1, keepdims=True))
    return ls.astype(np.float32)


# revision 30
# speedup vs baseline: 1.4894x; 1.0910x over previous
"""MoE-routing attention kernel for 8 Trainium2 NeuronCores — v2.

Expert parallelism (1 expert per core), full inputs in, full output out.
v2 restructures v1 around three ideas:

1. Merged q/k projection: S[s,t] = q_t.k_s = x_t M x_s + a[t] + c[s] + cc
   with M = Wq Wk^T precomputed ON DEVICE once per expert (dense 1024^3
   matmul that also warms the PE), a = x.(Wq bk), c = x.(Wk bq),
   cc = bq.bk. Per batch this is ONE projection G' = x_g M^T (64 MMs)
   plus 24 score MMs instead of two projections (128 MMs) + scores.
2. bf16 everywhere on the S path (validated: rel err stays at 1e-6
   because scores are divided by D=1024 before exp). bf16 halves
   LDWEIGHTS cost (FWL) and DMA volume.
3. Gathered x arrives TRANSPOSED directly via dma_gather(transpose=True)
   (bf16, one instruction per batch) — no PE transposes, no PSUM copies.
   Pad slots point at a host-appended zero row of x.

The gate runs with wg STATIONARY (8 LDWEIGHTS total instead of 256):
logits^T = wg^T @ x^T in [8, N] layout, then 8 tiny PE transposes per
batch back to [token, 8] for the top-2/softmax tail (fp32 - routing
flips are the one numerically fragile spot).

Weight-derived vectors (u = wv @ wo_rowsum, aw = Wq bk, cv = Wk bq and
scalars cc, c0 = bv.wo_rowsum, boS = sum bo) are folded on the host like
bias preprocessing. All model FLOPs (gate, M, G', S, attention combine)
stay on device.

Host: sums the 8 per-core [B,T] contribution vectors and applies the
final log_softmax (the SPMD combine/unshard step), as in v1.
"""

import sys

import numpy as np

for _p in ("/opt/trn_rl_repo", "/root/.axon_site/_ro/trn_rl_repo"):
    if _p not in sys.path:
        sys.path.append(_p)

import ml_dtypes  # noqa: E402

import concourse.bass as bass  # noqa: E402
import concourse.bass_isa as bass_isa  # noqa: E402
import concourse.mybir as mybir  # noqa: E402
import concourse.tile as tile  # noqa: E402
from concourse import bacc  # noqa: E402
from concourse import bass_utils  # noqa: E402
from concourse.bass import ts  # noqa: E402
from concourse.masks import make_identity  # noqa: E402

P = 128
B, T, D, E = 4, 1024, 1024, 8
DH = D
N = B * T
DC = D // P  # 8 contraction chunks
ST = T // P  # 8 token tiles per batch
CAP = 384  # gathered slot capacity per (expert, batch)
SC = CAP // P  # 3 slot tiles
NZ = N  # index of the host-appended all-zero x row (pad target)
BIG = 1 << 20
F32 = mybir.dt.float32
F32R = mybir.dt.float32r
BF16 = mybir.dt.bfloat16
I32 = mybir.dt.int32
I16 = mybir.dt.int16
AF = mybir.ActivationFunctionType
OP = mybir.AluOpType
AX = mybir.AxisListType
RED = bass_isa.ReduceOp

_CACHE = {}


TS = N // E  # tokens per core's gate shard


def _emit(nc, tc, dt_in, dt_out):
    (xTs, xnb, wg_d, wqT_d, wkT_d, w3_d, sconst_d) = dt_in
    (out_d,) = dt_out

    with tc.tile_pool(name="const", bufs=1) as const, tc.tile_pool(
        name="wqk", bufs=1
    ) as wqk, tc.tile_pool(name="msb", bufs=1) as msbp, tc.tile_pool(
        name="drams", bufs=1, space="DRAM"
    ) as dramp:
        # ---------------- constants ----------------
        wg_sb = const.tile([P, DC, E], F32)
        nc.sync.dma_start(wg_sb[:], wg_d.ap().rearrange("(c p) e -> p c e", p=P))
        w3_sb = const.tile([P, DC, 3], BF16)
        nc.sync.dma_start(w3_sb[:], w3_d.ap().rearrange("(c p) k -> p c k", p=P))
        bias3 = const.tile([3, 1], F32)
        nc.sync.dma_start(bias3[:], sconst_d.ap()[0:3, None])
        boS_bc = const.tile([P, 1], F32)
        nc.sync.dma_start(
            boS_bc[:], sconst_d.ap()[3:4][None, :].to_broadcast([P, 1])
        )

        idn = const.tile([P, P], F32)
        make_identity(nc, idn[:])
        ones_bf = const.tile([1, P], BF16)
        nc.vector.memset(ones_bf[:], 1.0)
        iosc_i = const.tile([P, SC], I32)  # value = slot j = c*128 + p
        nc.gpsimd.iota(iosc_i[:], pattern=[[P, SC]], base=0, channel_multiplier=1)
        iosc = const.tile([P, SC], F32)
        nc.vector.tensor_copy(iosc[:], iosc_i[:])
        tk16_i = const.tile([16, T // 16], I32)  # token id t = f*16 + p
        nc.gpsimd.iota(
            tk16_i[:], pattern=[[16, T // 16]], base=0, channel_multiplier=1
        )
        tk16 = const.tile([16, T // 16], F32)
        nc.vector.tensor_copy(tk16[:], tk16_i[:])
        zt = const.tile([P, ST], F32)
        nc.vector.memset(zt[:], 0.0)

        Msb = msbp.tile([P, DC, D], BF16)  # M^T[d', d]; chunk dc = d' rows

        sc_d = dramp.tile([N], F32, tag="scd", name="scd")
        a2a_in = dramp.tile([E, 2, TS], F32, tag="a2ain", name="a2ain")
        a2a_out = dramp.tile([E, 2, TS], F32, tag="a2aout", name="a2aout")
        iw_d = [
            dramp.tile([CAP], F32, tag=f"iwd{b}", name=f"iwd{b}")
            for b in range(B)
        ]
        cu_d = [
            dramp.tile([2, CAP], F32, tag=f"cud{b}", name=f"cud{b}")
            for b in range(B)
        ]

        with tc.tile_pool(name="pb", bufs=1) as pbp, tc.tile_pool(
            name="gsb", bufs=3
        ) as gsb, tc.tile_pool(name="gbig", bufs=2) as gbig, tc.tile_pool(
            name="psm", bufs=1, space="PSUM"
        ) as psm, tc.tile_pool(name="psg", bufs=1, space="PSUM") as psg:
            # per-batch persistent tiles
            cwb = [
                pbp.tile([P, ST], F32, tag=f"cwb{b}", name=f"cwb{b}")
                for b in range(B)
            ]
            idxt = [
                pbp.tile([P, SC], I32, tag=f"idxt{b}", name=f"idxt{b}")
                for b in range(B)
            ]
            idx16 = [
                pbp.tile([P, CAP // 16], I16, tag=f"idx16{b}", name=f"idx16{b}")
                for b in range(B)
            ]
            xgT = [
                pbp.tile([P, DC, CAP], BF16, tag=f"xgT{b}", name=f"xgT{b}")
                for b in range(B)
            ]
            omc = [
                pbp.tile([P, SC], F32, tag=f"omc{b}", name=f"omc{b}")
                for b in range(B)
            ]
            cspd = [
                pbp.tile([P, SC], F32, tag=f"cspd{b}", name=f"cspd{b}")
                for b in range(B)
            ]
            wv_w = [
                pbp.tile([P, SC], BF16, tag=f"wvw{b}", name=f"wvw{b}")
                for b in range(B)
            ]
            a_bf = [
                pbp.tile([1, CAP], BF16, tag=f"abf{b}", name=f"abf{b}")
                for b in range(B)
            ]

            def m_build(fcs):
                """M^T[d'-tile, d] = sum_f WkT[f, d']^T WqT[f, d] for d'-tiles
                in fcs. Each group accumulates over all 8 f chunks."""
                for dt in fcs:
                    for half in range(2):
                        pm = psm.tile(
                            [P, 512], F32, tag="m", bufs=2,
                            name=f"m{dt}_{half}",
                        )
                        for fc in range(DC):
                            nc.tensor.matmul(
                                pm[:],
                                wk_sb[:, fc, ts(dt, P)],
                                wq_sb[:, fc, ts(half, 512)],
                                start=(fc == 0),
                                stop=(fc == DC - 1),
                            )
                        nc.scalar.activation(
                            Msb[:, dt, ts(half, 512)], pm[:], AF.Copy
                        )

            def gate():
                """Sharded gate: this core computes logits for its own
                TS-token slice (full fp32), top-2 + softmax for ALL experts
                in [8, TS] layout, then AllToAll so every core ends up with
                its own expert's mask/cw over all N tokens."""
                xts = pbp.tile([P, DC, TS], F32, tag="xts", name="xts")
                nc.sync.dma_start(
                    xts[:], xTs.ap().rearrange("(c p) n -> p c n", p=P)
                )
                pg = psm.tile([P, 512], F32, tag="m", bufs=2, name="pgate")
                for dc in range(DC):
                    nc.tensor.matmul(
                        pg[0:E, 0:TS],
                        wg_sb[:, dc],
                        xts[:, dc],
                        start=(dc == 0),
                        stop=(dc == DC - 1),
                    )
                lsh = gsb.tile([E, TS], F32, tag="lsh")
                nc.scalar.activation(lsh[:], pg[0:E, 0:TS], AF.Copy)
                # transpose each 128-token tile to [token, E] and run the
                # top-2/softmax tail for ALL experts (PE/DVE/ACT only)
                mk4 = gsb.tile([P, TS // P, E], F32, tag="mk4")
                cw4 = gsb.tile([P, TS // P, E], F32, tag="cw4")
                for tt in range(TS // P):
                    tp = psg.tile(
                        [P, 512], F32, tag="g", bufs=3, name=f"tp{tt}"
                    )
                    nc.tensor.transpose(
                        tp[:, 0:E], lsh[:, ts(tt, P)], idn[0:E, 0:E]
                    )
                    gl = gsb.tile([P, E], F32, tag="gl")
                    nc.scalar.activation(gl[:], tp[:, 0:E], AF.Copy)
                    mx8 = gsb.tile([P, 8], F32, tag="mx8")
                    nc.vector.max(out=mx8[:], in_=gl[:])
                    mxn = gsb.tile([P, 1], F32, tag="mxn")
                    nc.vector.tensor_scalar_mul(mxn[:], mx8[:, 0:1], -1.0)
                    probs = gsb.tile([P, E], F32, tag="probs")
                    se = gsb.tile([P, 1], F32, tag="se")
                    nc.scalar.activation(
                        probs[:], gl[:], AF.Exp, bias=mxn[:], scale=1.0,
                        accum_out=se[:],
                    )
                    rs = gsb.tile([P, 1], F32, tag="rs")
                    nc.vector.reciprocal(rs[:], se[:])
                    nc.vector.tensor_scalar(
                        mk4[:, tt, :], gl[:], mx8[:, 1:2], None, op0=OP.is_ge
                    )
                    nc.vector.scalar_tensor_tensor(
                        cw4[:, tt, :], probs[:], rs[:], mk4[:, tt, :],
                        op0=OP.mult, op1=OP.mult,
                    )
                # transpose back to [expert, token] for the AllToAll pack
                mkE = gsb.tile([E, TS], F32, tag="mkE")
                cwE = gsb.tile([E, TS], F32, tag="cwE")
                for tt in range(TS // P):
                    for src, dst in ((mk4, mkE), (cw4, cwE)):
                        tq = psg.tile(
                            [P, 512], F32, tag="g", bufs=3,
                            name=f"tq{tt}_{dst.name}",
                        )
                        nc.tensor.transpose(
                            tq[0:E, 0:P], src[:, tt, :], idn[:]
                        )
                        nc.scalar.activation(
                            dst[:, ts(tt, P)], tq[0:E, 0:P], AF.Copy
                        )
                nc.sync.dma_start(a2a_in[:, 0, :], mkE[:])
                nc.sync.dma_start(a2a_in[:, 1, :], cwE[:])
                nc.gpsimd.collective_compute(
                    "AllToAll",
                    mybir.AluOpType.bypass,
                    replica_groups=[list(range(E))],
                    ins=[a2a_in[:]],
                    outs=[a2a_out[:]],
                )
                for b in range(B):
                    for r in range(2):
                        nc.sync.dma_start(
                            cwb[b][:, 4 * r : 4 * (r + 1)],
                            a2a_out[2 * b + r, 1, :].rearrange(
                                "(c p) -> p c", p=P
                            ),
                        )

            vals4 = [
                pbp.tile(
                    [16, T // 16 + SC * 8], F32, tag=f"vals{b}", name=f"vals{b}"
                )
                for b in range(B)
            ]
            idxw4 = [
                pbp.tile([16, CAP // 16], F32, tag=f"idxw{b}", name=f"idxw{b}")
                for b in range(B)
            ]
            nfu4 = [
                pbp.tile([1, 1], mybir.dt.uint32, tag=f"nfu{b}", name=f"nfu{b}")
                for b in range(B)
            ]
            cf4 = [
                pbp.tile([P, 1], F32, tag=f"cf{b}", name=f"cf{b}")
                for b in range(B)
            ]

            def gather1(b):
                """Per-token values (global token id if routed else -1) in a
                16-partition wrap, with CAP appended entries pointing at the
                zero x row so pads compact in behind the routed tokens."""
                vals = vals4[b]
                nc.vector.memset(vals[:, T // 16 :], float(NZ))
                for r in range(2):
                    nc.sync.dma_start(
                        vals[:, 32 * r : 32 * (r + 1)],
                        a2a_out[2 * b + r, 0, :].rearrange(
                            "(f p) -> p f", p=16
                        ),
                    )
                t1 = gsb.tile([16, T // 16], F32, tag="t1")
                nc.vector.tensor_scalar(
                    t1[:], tk16[:], float(b * T + 1), None, op0=OP.add
                )
                nc.vector.tensor_mul(t1[:], t1[:], vals[:, 0 : T // 16])
                nc.vector.tensor_scalar(
                    vals[:, 0 : T // 16], t1[:], -1.0, None, op0=OP.add
                )

            def gather2(b):
                """cf = C, int16 idx list + per-group replicas, slot->token
                int32 list via a DRAM bounce."""
                nfb = gsb.tile([P, 1], mybir.dt.uint32, tag="nfb")
                nc.gpsimd.partition_broadcast(nfb[:], nfu4[b][:], channels=P)
                nc.vector.tensor_copy(cf4[b][:], nfb[:])
                nc.vector.tensor_scalar(
                    cf4[b][:], cf4[b][:], float(-CAP), None, op0=OP.add
                )
                nc.vector.tensor_copy(idx16[b][0:16, :], idxw4[b][:])
                for r in range(1, P // 16):
                    nc.sync.dma_start(
                        idx16[b][16 * r : 16 * (r + 1), :], idx16[b][0:16, :]
                    )
                nc.sync.dma_start(
                    iw_d[b].rearrange("(f p) -> p f", p=16), idxw4[b][:]
                )
                idxtf = gsb.tile([P, SC], F32, tag="idxtf")
                nc.sync.dma_start(
                    idxtf[:], iw_d[b].rearrange("(c p) -> p c", p=P)
                )
                nc.vector.tensor_copy(idxt[b][:], idxtf[:])

            def gather3(b):
                """omega weights and the a/c/vw vectors."""
                cf = cf4[b]
                # omega: 1 for j < C, (T - C) at j == CAP-1, else 0
                tmc = gsb.tile([P, 1], F32, tag="tmc")
                nc.vector.tensor_scalar(
                    tmc[:], cf[:], -1.0, float(T), op0=OP.mult, op1=OP.add
                )
                rep = gsb.tile([P, SC], F32, tag="rep")
                nc.vector.tensor_scalar(
                    omc[b][:], iosc[:], cf[:], None, op0=OP.is_lt
                )
                nc.vector.tensor_scalar(
                    rep[:], iosc[:], float(CAP - 1), None, op0=OP.is_equal
                )
                nc.vector.tensor_scalar(rep[:], rep[:], tmc[:], None, op0=OP.mult)
                nc.vector.tensor_add(omc[b][:], omc[b][:], rep[:])

                # --- a, c, vw vectors: [3, CAP] = w3^T x_g^T (+ biases) ---
                p3 = psg.tile([P, 512], F32, tag="g", bufs=3, name=f"p3{b}")
                for dc in range(DC):
                    nc.tensor.matmul(
                        p3[0:3, 0:CAP],
                        w3_sb[:, dc],
                        xgT[b][:, dc],
                        start=(dc == 0),
                        stop=(dc == DC - 1),
                    )
                avc = gsb.tile([3, CAP], F32, tag="avc")
                nc.scalar.activation(
                    avc[:], p3[0:3, 0:CAP], AF.Identity, bias=bias3[:]
                )
                nc.vector.tensor_copy(a_bf[b][:], avc[0:1, :])
                nc.sync.dma_start(cu_d[b][:], avc[1:3, :])
                nc.sync.dma_start(
                    cspd[b][:], cu_d[b][0].rearrange("(c p) -> p c", p=P)
                )
                vwsp = gsb.tile([P, SC], F32, tag="vwsp")
                nc.sync.dma_start(
                    vwsp[:], cu_d[b][1].rearrange("(c p) -> p c", p=P)
                )
                nc.vector.tensor_scalar_mul(cspd[b][:], cspd[b][:], 1.0 / D)
                vww = gsb.tile([P, SC], F32, tag="vww")
                nc.vector.tensor_mul(vww[:], vwsp[:], omc[b][:])
                nc.vector.tensor_copy(wv_w[b][:], vww[:])

            def block(b):
                """Per-batch dense compute: G' = x_g M^T, S, exp, Z, num,
                combine."""
                # --- G'^T[d, s] = sum_d' M^T[d', d] xgT[d', s] ---
                Gsb = gbig.tile([P, DC, CAP], BF16, tag="Gsb", name=f"Gsb{b}")
                for dt in range(DC):
                    pgt = psg.tile(
                        [P, 512], F32, tag="g", bufs=3, name=f"G{b}_{dt}"
                    )
                    for dc in range(DC):
                        nc.tensor.matmul(
                            pgt[:, 0:CAP],
                            Msb[:, dc, ts(dt, P)],
                            xgT[b][:, dc],
                            start=(dc == 0),
                            stop=(dc == DC - 1),
                        )
                    nc.scalar.activation(Gsb[:, dt], pgt[:, 0:CAP], AF.Copy)

                # --- S[s, t] = sum_d G'^T[d, s] xgT[d, t] + a[t] (+c[s]/D
                # via exp bias); E = exp(S/D) with free-axis row sums ---
                Eg = gbig.tile([P, SC, CAP], BF16, tag="Eg", name=f"Eg{b}")
                eacc = gsb.tile([P, SC], F32, tag="eacc")
                erw = gsb.tile([P, SC], F32, tag="erw")
                for st in range(SC):
                    pss = psg.tile(
                        [P, 512], F32, tag="g", bufs=3, name=f"S{b}_{st}"
                    )
                    for dt in range(DC):
                        nc.tensor.matmul(
                            pss[:, 0:CAP],
                            Gsb[:, dt, ts(st, P)],
                            xgT[b][:, dt],
                            start=(dt == 0),
                            stop=False,
                        )
                    nc.tensor.matmul(
                        pss[:, 0:CAP],
                        ones_bf[:],
                        a_bf[b][:],
                        start=False,
                        stop=True,
                    )
                    nc.scalar.activation(
                        Eg[:, st], pss[:, 0:CAP], AF.Exp,
                        bias=cspd[b][:, st : st + 1], scale=float(1.0 / D),
                        accum_out=eacc[:, st : st + 1],
                    )
                    nc.vector.tensor_scalar(
                        erw[:, st : st + 1],
                        Eg[:, st, CAP - 1 : CAP],
                        float(T - CAP),
                        eacc[:, st : st + 1],
                        op0=OP.mult,
                        op1=OP.add,
                    )
                # Z = omega . erw (both [slot] in partition-chunk layout)
                scr3 = gsb.tile([P, SC], F32, tag="scr3")
                zp = gsb.tile([P, 1], F32, tag="zp")
                nc.vector.tensor_mul(scr3[:], erw[:], omc[b][:])
                nc.vector.reduce_sum(zp[:], scr3[:], axis=AX.X)
                za = gsb.tile([P, 1], F32, tag="za")
                nc.gpsimd.partition_all_reduce(
                    za[:], zp[:], channels=P, reduce_op=RED.add
                )
                rZ = gsb.tile([P, 1], F32, tag="rZ")
                nc.vector.reciprocal(rZ[:], za[:])
                # num[t] = sum_s E[s, t] (omega_s vw[s])
                numg = gsb.tile([P, SC], F32, tag="numg")
                for ti in range(SC):
                    pn = psg.tile(
                        [P, 512], F32, tag="g", bufs=3, name=f"pn{b}_{ti}"
                    )
                    for sc in range(SC):
                        nc.tensor.matmul(
                            pn[:, 0:1],
                            Eg[:, sc, ts(ti, P)],
                            wv_w[b][:, sc : sc + 1],
                            start=(sc == 0),
                            stop=(sc == SC - 1),
                        )
                    nc.scalar.activation(numg[:, ti : ti + 1], pn[:, 0:1], AF.Copy)
                # out_sum = num/Z + boS; scatter to token space
                outg = gsb.tile([P, SC], F32, tag="outg")
                nc.vector.tensor_scalar(
                    outg[:], numg[:], rZ[:], boS_bc[:], op0=OP.mult, op1=OP.add
                )
                for i in range(SC):
                    nc.gpsimd.indirect_dma_start(
                        out=sc_d[:, None],
                        out_offset=bass.IndirectOffsetOnAxis(
                            ap=idxt[b][:, i : i + 1], axis=0
                        ),
                        in_=outg[:, i : i + 1],
                        in_offset=None,
                        bounds_check=N - 1,
                        oob_is_err=False,
                    )
                scb = gsb.tile([P, ST], F32, tag="scb")
                nc.sync.dma_start(
                    scb[:],
                    sc_d[b * T : (b + 1) * T].rearrange("(c p) -> p c", p=P),
                )
                ob = gsb.tile([P, ST], F32, tag="ob")
                nc.vector.tensor_mul(ob[:], scb[:], cwb[b][:])
                nc.sync.dma_start(out_d.ap()[b], ob[:])

            # ---------------- pipeline ----------------
            wq_sb = wqk.tile([P, DC, D], BF16, tag="wq", name="wq")
            wk_sb = wqk.tile([P, DC, D], BF16, tag="wk", name="wk")
            nc.sync.dma_start(
                wq_sb[:], wqT_d.ap().rearrange("(c p) d -> p c d", p=P)
            )
            nc.sync.dma_start(
                wk_sb[:], wkT_d.ap().rearrange("(c p) d -> p c d", p=P)
            )
            # zero the token-space scatter target once
            for bb in range(B):
                nc.sync.dma_start(
                    sc_d[bb * T : (bb + 1) * T].rearrange("(c p) -> p c", p=P),
                    zt[:],
                )
            gate()
            for b in range(B):
                gather1(b)
            for b in range(B):
                nc.gpsimd.sparse_gather(
                    idxw4[b][:], vals4[b][:], num_found=nfu4[b][:]
                )
            m_build(range(DC))
            for b in range(B):
                gather2(b)
            for b in range(B):
                nc.gpsimd.dma_gather(
                    xgT[b][:],
                    xnb.ap(),
                    idx16[b][:],
                    num_idxs=CAP,
                    num_idxs_reg=CAP,
                    elem_size=D,
                    transpose=True,
                )
            for b in range(B):
                gather3(b)
            block(0)
            block(1)
            block(2)
            block(3)


def build_nc():
    nc = bacc.Bacc("TRN2", target_bir_lowering=False, debug=False, num_devices=8)
    xTs = nc.dram_tensor("xTs", [D, TS], F32, kind="ExternalInput")
    xnb = nc.dram_tensor("xnb", [N + 1, D], BF16, kind="ExternalInput")
    wg_d = nc.dram_tensor("wg", [D, E], F32, kind="ExternalInput")
    wqT_d = nc.dram_tensor("wqT", [DH, D], BF16, kind="ExternalInput")
    wkT_d = nc.dram_tensor("wkT", [DH, D], BF16, kind="ExternalInput")
    w3_d = nc.dram_tensor("w3", [D, 3], BF16, kind="ExternalInput")
    sconst_d = nc.dram_tensor("sconst", [8], F32, kind="ExternalInput")
    out_d = nc.dram_tensor("contrib", [B, P, ST], F32, kind="ExternalOutput")
    with tile.TileContext(nc) as tc:
        _emit(
            nc,
            tc,
            (xTs, xnb, wg_d, wqT_d, wkT_d, w3_d, sconst_d),
            (out_d,),
        )
    nc.compile()
    return nc


def make_in_maps(x, wg, wqkv, bqkv, wo, bo):
    xn = np.ascontiguousarray(x.reshape(N, D), dtype=np.float32)
    xT = np.ascontiguousarray(xn.T)
    wg32 = np.ascontiguousarray(wg, dtype=np.float32)
    xnb = np.zeros((N + 1, D), dtype=ml_dtypes.bfloat16)
    xnb[:N] = xn
    in_maps = []
    for e in range(E):
        wq = wqkv[e][:, 0::3].astype(np.float64)
        wk = wqkv[e][:, 1::3].astype(np.float64)
        wv = wqkv[e][:, 2::3].astype(np.float64)
        bq = bqkv[e][0::3].astype(np.float64)
        bk = bqkv[e][1::3].astype(np.float64)
        bv = bqkv[e][2::3].astype(np.float64)
        wos = wo[e].astype(np.float64).sum(axis=1)
        u = wv @ wos
        aw = wq @ bk
        cv = wk @ bq
        cc = float(bq @ bk)
        c0 = float(bv @ wos)
        boS = float(bo[e].astype(np.float64).sum())
        in_maps.append(
            {
                "xTs": np.ascontiguousarray(xT[:, e * TS : (e + 1) * TS]),
                "xnb": xnb,
                "wg": wg32,
                "wqT": np.ascontiguousarray(wq.T).astype(ml_dtypes.bfloat16),
                "wkT": np.ascontiguousarray(wk.T).astype(ml_dtypes.bfloat16),
                "w3": np.ascontiguousarray(
                    np.stack([aw, cv, u], axis=1)
                ).astype(ml_dtypes.bfloat16),
                "sconst": np.array(
                    [cc, 0.0, c0, boS, 0.0, 0.0, 0.0, 0.0], dtype=np.float32
                ),
            }
        )
    return in_maps


def run_device(in_maps, trace=False):
    if "nc" not in _CACHE:
        _CACHE["nc"] = build_nc()
    return bass_utils.run_bass_kernel_spmd(
        _CACHE["nc"], in_maps, core_ids=list(range(E)), trace=trace
    )


def kernel(x, wg, wqkv, bqkv, wo, bo, top_k):
    assert int(top_k) == 2, f"kernel hardcodes top_k=2, got {top_k}"
    x = np.asarray(x, np.float32)
    wg = np.asarray(wg, np.float32)
    wqkv = np.asarray(wqkv, np.float32)
    bqkv = np.asarray(bqkv, np.float32)
    wo = np.asarray(wo, np.float32)
    bo = np.asarray(bo, np.float32)

    res = run_device(make_in_maps(x, wg, wqkv, bqkv, wo, bo))
    total = np.zeros((B, T), np.float64)
    for c in range(E):
        contrib = res.results[c]["contrib"]  # [B, P, ST], t = tt*128 + p
        total += contrib.transpose(0, 2, 1).reshape(B, T).astype(np.float64)
    m = total.max(axis=1, keepdims=True)
    ls = total - m - np.log(np.exp(total - m).sum(axis=1, keepdims=True))
    return ls.astype(np.float32)


# revision 31
# speedup vs baseline: 1.5393x; 1.0335x over previous
"""MoE-routing attention kernel for 8 Trainium2 NeuronCores — v2.

Expert parallelism (1 expert per core), full inputs in, full output out.
v2 restructures v1 around three ideas:

1. Merged q/k projection: S[s,t] = q_t.k_s = x_t M x_s + a[t] + c[s] + cc
   with M = Wq Wk^T precomputed ON DEVICE once per expert (dense 1024^3
   matmul that also warms the PE), a = x.(Wq bk), c = x.(Wk bq),
   cc = bq.bk. Per batch this is ONE projection G' = x_g M^T (64 MMs)
   plus 24 score MMs instead of two projections (128 MMs) + scores.
2. bf16 everywhere on the S path (validated: rel err stays at 1e-6
   because scores are divided by D=1024 before exp). bf16 halves
   LDWEIGHTS cost (FWL) and DMA volume.
3. Gathered x arrives TRANSPOSED directly via dma_gather(transpose=True)
   (bf16, one instruction per batch) — no PE transposes, no PSUM copies.
   Pad slots point at a host-appended zero row of x.

The gate runs with wg STATIONARY (8 LDWEIGHTS total instead of 256):
logits^T = wg^T @ x^T in [8, N] layout, then 8 tiny PE transposes per
batch back to [token, 8] for the top-2/softmax tail (fp32 - routing
flips are the one numerically fragile spot).

Weight-derived vectors (u = wv @ wo_rowsum, aw = Wq bk, cv = Wk bq and
scalars cc, c0 = bv.wo_rowsum, boS = sum bo) are folded on the host like
bias preprocessing. All model FLOPs (gate, M, G', S, attention combine)
stay on device.

Host: sums the 8 per-core [B,T] contribution vectors and applies the
final log_softmax (the SPMD combine/unshard step), as in v1.
"""

import sys

import numpy as np

for _p in ("/opt/trn_rl_repo", "/root/.axon_site/_ro/trn_rl_repo"):
    if _p not in sys.path:
        sys.path.append(_p)

import ml_dtypes  # noqa: E402

import concourse.bass as bass  # noqa: E402
import concourse.bass_isa as bass_isa  # noqa: E402
import concourse.mybir as mybir  # noqa: E402
import concourse.tile as tile  # noqa: E402
from concourse import bacc  # noqa: E402
from concourse import bass_utils  # noqa: E402
from concourse.bass import ts  # noqa: E402
from concourse.masks import make_identity  # noqa: E402

P = 128
B, T, D, E = 4, 1024, 1024, 8
DH = D
N = B * T
DC = D // P  # 8 contraction chunks
ST = T // P  # 8 token tiles per batch
CAP = 384  # gathered slot capacity per (expert, batch)
SC = CAP // P  # 3 slot tiles
NZ = N  # index of the host-appended all-zero x row (pad target)
BIG = 1 << 20
F32 = mybir.dt.float32
F32R = mybir.dt.float32r
BF16 = mybir.dt.bfloat16
I32 = mybir.dt.int32
I16 = mybir.dt.int16
AF = mybir.ActivationFunctionType
OP = mybir.AluOpType
AX = mybir.AxisListType
RED = bass_isa.ReduceOp

_CACHE = {}


TS = N // E  # tokens per core's gate shard


def _emit(nc, tc, dt_in, dt_out):
    (xTs, xnb, wg_d, wqT_d, wkT_d, w3_d, sconst_d) = dt_in
    (out_d,) = dt_out

    with tc.tile_pool(name="const", bufs=1) as const, tc.tile_pool(
        name="wqk", bufs=1
    ) as wqk, tc.tile_pool(name="msb", bufs=1) as msbp, tc.tile_pool(
        name="drams", bufs=1, space="DRAM"
    ) as dramp:
        # ---------------- constants ----------------
        wg_sb = const.tile([P, DC, E], F32)
        nc.sync.dma_start(wg_sb[:], wg_d.ap().rearrange("(c p) e -> p c e", p=P))
        w3_sb = const.tile([P, DC, 3], BF16)
        nc.sync.dma_start(w3_sb[:], w3_d.ap().rearrange("(c p) k -> p c k", p=P))
        bias3 = const.tile([3, 1], F32)
        nc.sync.dma_start(bias3[:], sconst_d.ap()[0:3, None])
        boS_bc = const.tile([P, 1], F32)
        nc.sync.dma_start(
            boS_bc[:], sconst_d.ap()[3:4][None, :].to_broadcast([P, 1])
        )

        idn = const.tile([P, P], F32)
        make_identity(nc, idn[:])
        ones_bf = const.tile([1, P], BF16)
        nc.vector.memset(ones_bf[:], 1.0)
        iosc_i = const.tile([P, SC], I32)  # value = slot j = c*128 + p
        nc.gpsimd.iota(iosc_i[:], pattern=[[P, SC]], base=0, channel_multiplier=1)
        iosc = const.tile([P, SC], F32)
        nc.vector.tensor_copy(iosc[:], iosc_i[:])
        tk16_i = const.tile([16, T // 16], I32)  # token id t = f*16 + p
        nc.gpsimd.iota(
            tk16_i[:], pattern=[[16, T // 16]], base=0, channel_multiplier=1
        )
        tk16 = const.tile([16, T // 16], F32)
        nc.vector.tensor_copy(tk16[:], tk16_i[:])
        zt = const.tile([P, ST], F32)
        nc.vector.memset(zt[:], 0.0)

        Msb = msbp.tile([P, DC, D], BF16)  # M^T[d', d]; chunk dc = d' rows

        sc_d = dramp.tile([N], F32, tag="scd", name="scd")
        a2a_in = dramp.tile([E, 2, TS], F32, tag="a2ain", name="a2ain")
        a2a_out = dramp.tile([E, 2, TS], F32, tag="a2aout", name="a2aout")
        iw_d = [
            dramp.tile([CAP], F32, tag=f"iwd{b}", name=f"iwd{b}")
            for b in range(B)
        ]
        nf_d = [
            dramp.tile([1], mybir.dt.uint32, tag=f"nfd{b}", name=f"nfd{b}")
            for b in range(B)
        ]
        cu_d = [
            dramp.tile([2, CAP], F32, tag=f"cud{b}", name=f"cud{b}")
            for b in range(B)
        ]

        with tc.tile_pool(name="pb", bufs=1) as pbp, tc.tile_pool(
            name="gsb", bufs=3
        ) as gsb, tc.tile_pool(name="gbig", bufs=2) as gbig, tc.tile_pool(
            name="psm", bufs=1, space="PSUM"
        ) as psm, tc.tile_pool(name="psg", bufs=1, space="PSUM") as psg:
            # per-batch persistent tiles
            cwb = [
                pbp.tile([P, ST], F32, tag=f"cwb{b}", name=f"cwb{b}")
                for b in range(B)
            ]
            idxt = [
                pbp.tile([P, SC], I32, tag=f"idxt{b}", name=f"idxt{b}")
                for b in range(B)
            ]
            idx16 = [
                pbp.tile([P, CAP // 16], I16, tag=f"idx16{b}", name=f"idx16{b}")
                for b in range(B)
            ]
            xgT = [
                pbp.tile([P, DC, CAP], BF16, tag=f"xgT{b}", name=f"xgT{b}")
                for b in range(B)
            ]
            omc = [
                pbp.tile([P, SC], F32, tag=f"omc{b}", name=f"omc{b}")
                for b in range(B)
            ]
            cspd = [
                pbp.tile([P, SC], F32, tag=f"cspd{b}", name=f"cspd{b}")
                for b in range(B)
            ]
            wv_w = [
                pbp.tile([P, SC], BF16, tag=f"wvw{b}", name=f"wvw{b}")
                for b in range(B)
            ]
            a_bf = [
                pbp.tile([1, CAP], BF16, tag=f"abf{b}", name=f"abf{b}")
                for b in range(B)
            ]

            def m_build(fcs):
                """M^T[d'-tile, d] = sum_f WkT[f, d']^T WqT[f, d] for d'-tiles
                in fcs. Each group accumulates over all 8 f chunks."""
                for dt in fcs:
                    for half in range(2):
                        pm = psm.tile(
                            [P, 512], F32, tag="m", bufs=2,
                            name=f"m{dt}_{half}",
                        )
                        for fc in range(DC):
                            nc.tensor.matmul(
                                pm[:],
                                wk_sb[:, fc, ts(dt, P)],
                                wq_sb[:, fc, ts(half, 512)],
                                start=(fc == 0),
                                stop=(fc == DC - 1),
                            )
                        nc.scalar.activation(
                            Msb[:, dt, ts(half, 512)], pm[:], AF.Copy
                        )

            def gate():
                """Sharded gate: this core computes logits for its own
                TS-token slice (full fp32), top-2 + softmax for ALL experts
                in [8, TS] layout, then AllToAll so every core ends up with
                its own expert's mask/cw over all N tokens."""
                xts = pbp.tile([P, DC, TS], F32, tag="xts", name="xts")
                nc.sync.dma_start(
                    xts[:], xTs.ap().rearrange("(c p) n -> p c n", p=P)
                )
                pg = psm.tile([P, 512], F32, tag="m", bufs=2, name="pgate")
                for dc in range(DC):
                    nc.tensor.matmul(
                        pg[0:E, 0:TS],
                        wg_sb[:, dc],
                        xts[:, dc],
                        start=(dc == 0),
                        stop=(dc == DC - 1),
                    )
                lsh = gsb.tile([E, TS], F32, tag="lsh")
                nc.scalar.activation(lsh[:], pg[0:E, 0:TS], AF.Copy)
                # transpose each 128-token tile to [token, E] and run the
                # top-2/softmax tail for ALL experts (PE/DVE/ACT only)
                mk4 = gsb.tile([P, TS // P, E], F32, tag="mk4")
                cw4 = gsb.tile([P, TS // P, E], F32, tag="cw4")
                for tt in range(TS // P):
                    tp = psg.tile(
                        [P, 512], F32, tag="g", bufs=3, name=f"tp{tt}"
                    )
                    nc.tensor.transpose(
                        tp[:, 0:E], lsh[:, ts(tt, P)], idn[0:E, 0:E]
                    )
                    gl = gsb.tile([P, E], F32, tag="gl")
                    nc.scalar.activation(gl[:], tp[:, 0:E], AF.Copy)
                    mx8 = gsb.tile([P, 8], F32, tag="mx8")
                    nc.vector.max(out=mx8[:], in_=gl[:])
                    mxn = gsb.tile([P, 1], F32, tag="mxn")
                    nc.vector.tensor_scalar_mul(mxn[:], mx8[:, 0:1], -1.0)
                    probs = gsb.tile([P, E], F32, tag="probs")
                    se = gsb.tile([P, 1], F32, tag="se")
                    nc.scalar.activation(
                        probs[:], gl[:], AF.Exp, bias=mxn[:], scale=1.0,
                        accum_out=se[:],
                    )
                    rs = gsb.tile([P, 1], F32, tag="rs")
                    nc.vector.reciprocal(rs[:], se[:])
                    nc.vector.tensor_scalar(
                        mk4[:, tt, :], gl[:], mx8[:, 1:2], None, op0=OP.is_ge
                    )
                    nc.vector.scalar_tensor_tensor(
                        cw4[:, tt, :], probs[:], rs[:], mk4[:, tt, :],
                        op0=OP.mult, op1=OP.mult,
                    )
                # transpose back to [expert, token] for the AllToAll pack
                mkE = gsb.tile([E, TS], F32, tag="mkE")
                cwE = gsb.tile([E, TS], F32, tag="cwE")
                for tt in range(TS // P):
                    for src, dst in ((mk4, mkE), (cw4, cwE)):
                        tq = psg.tile(
                            [P, 512], F32, tag="g", bufs=3,
                            name=f"tq{tt}_{dst.name}",
                        )
                        nc.tensor.transpose(
                            tq[0:E, 0:P], src[:, tt, :], idn[:]
                        )
                        nc.scalar.activation(
                            dst[:, ts(tt, P)], tq[0:E, 0:P], AF.Copy
                        )
                nc.sync.dma_start(a2a_in[:, 0, :], mkE[:])
                nc.sync.dma_start(a2a_in[:, 1, :], cwE[:])
                nc.gpsimd.collective_compute(
                    "AllToAll",
                    mybir.AluOpType.bypass,
                    replica_groups=[list(range(E))],
                    ins=[a2a_in[:]],
                    outs=[a2a_out[:]],
                )
                for b in range(B):
                    for r in range(2):
                        nc.sync.dma_start(
                            cwb[b][:, 4 * r : 4 * (r + 1)],
                            a2a_out[2 * b + r, 1, :].rearrange(
                                "(c p) -> p c", p=P
                            ),
                        )

            vals4 = [
                pbp.tile(
                    [16, T // 16 + SC * 8], F32, tag=f"vals{b}", name=f"vals{b}"
                )
                for b in range(B)
            ]
            idxw4 = [
                pbp.tile([16, CAP // 16], F32, tag=f"idxw{b}", name=f"idxw{b}")
                for b in range(B)
            ]
            nfu4 = [
                pbp.tile([1, 1], mybir.dt.uint32, tag=f"nfu{b}", name=f"nfu{b}")
                for b in range(B)
            ]
            cf4 = [
                pbp.tile([P, 1], F32, tag=f"cf{b}", name=f"cf{b}")
                for b in range(B)
            ]

            def gather1(b):
                """Per-token values (global token id if routed else -1) in a
                16-partition wrap, with CAP appended entries pointing at the
                zero x row so pads compact in behind the routed tokens."""
                vals = vals4[b]
                nc.vector.memset(vals[:, T // 16 :], float(NZ))
                for r in range(2):
                    nc.sync.dma_start(
                        vals[:, 32 * r : 32 * (r + 1)],
                        a2a_out[2 * b + r, 0, :].rearrange(
                            "(f p) -> p f", p=16
                        ),
                    )
                t1 = gsb.tile([16, T // 16], F32, tag="t1")
                nc.vector.tensor_scalar(
                    t1[:], tk16[:], float(b * T + 1), None, op0=OP.add
                )
                nc.vector.tensor_mul(t1[:], t1[:], vals[:, 0 : T // 16])
                nc.vector.tensor_scalar(
                    vals[:, 0 : T // 16], t1[:], -1.0, None, op0=OP.add
                )

            def gather2(b):
                """cf = C, int16 idx list + per-group replicas, slot->token
                int32 list via a DRAM bounce."""
                nc.sync.dma_start(nf_d[b][:, None], nfu4[b][:])
                nfb = gsb.tile([P, 1], mybir.dt.uint32, tag="nfb")
                nc.sync.dma_start(
                    nfb[:], nf_d[b][None, :].to_broadcast([P, 1])
                )
                nc.vector.tensor_copy(cf4[b][:], nfb[:])
                nc.vector.tensor_scalar(
                    cf4[b][:], cf4[b][:], float(-CAP), None, op0=OP.add
                )
                nc.vector.tensor_copy(idx16[b][0:16, :], idxw4[b][:])
                for r in range(1, P // 16):
                    nc.sync.dma_start(
                        idx16[b][16 * r : 16 * (r + 1), :], idx16[b][0:16, :]
                    )
                nc.sync.dma_start(
                    iw_d[b].rearrange("(f p) -> p f", p=16), idxw4[b][:]
                )
                idxtf = gsb.tile([P, SC], F32, tag="idxtf")
                nc.sync.dma_start(
                    idxtf[:], iw_d[b].rearrange("(c p) -> p c", p=P)
                )
                nc.vector.tensor_copy(idxt[b][:], idxtf[:])

            def gather3(b):
                """omega weights and the a/c/vw vectors."""
                cf = cf4[b]
                # omega: 1 for j < C, (T - C) at j == CAP-1, else 0
                tmc = gsb.tile([P, 1], F32, tag="tmc")
                nc.vector.tensor_scalar(
                    tmc[:], cf[:], -1.0, float(T), op0=OP.mult, op1=OP.add
                )
                rep = gsb.tile([P, SC], F32, tag="rep")
                nc.vector.tensor_scalar(
                    omc[b][:], iosc[:], cf[:], None, op0=OP.is_lt
                )
                nc.vector.tensor_scalar(
                    rep[:], iosc[:], float(CAP - 1), None, op0=OP.is_equal
                )
                nc.vector.tensor_scalar(rep[:], rep[:], tmc[:], None, op0=OP.mult)
                nc.vector.tensor_add(omc[b][:], omc[b][:], rep[:])

                # --- a, c, vw vectors: [3, CAP] = w3^T x_g^T (+ biases) ---
                p3 = psg.tile([P, 512], F32, tag="g", bufs=3, name=f"p3{b}")
                for dc in range(DC):
                    nc.tensor.matmul(
                        p3[0:3, 0:CAP],
                        w3_sb[:, dc],
                        xgT[b][:, dc],
                        start=(dc == 0),
                        stop=(dc == DC - 1),
                    )
                avc = gsb.tile([3, CAP], F32, tag="avc")
                nc.scalar.activation(
                    avc[:], p3[0:3, 0:CAP], AF.Identity, bias=bias3[:]
                )
                nc.vector.tensor_copy(a_bf[b][:], avc[0:1, :])
                nc.sync.dma_start(cu_d[b][:], avc[1:3, :])
                nc.sync.dma_start(
                    cspd[b][:], cu_d[b][0].rearrange("(c p) -> p c", p=P)
                )
                vwsp = gsb.tile([P, SC], F32, tag="vwsp")
                nc.sync.dma_start(
                    vwsp[:], cu_d[b][1].rearrange("(c p) -> p c", p=P)
                )
                nc.vector.tensor_scalar_mul(cspd[b][:], cspd[b][:], 1.0 / D)
                vww = gsb.tile([P, SC], F32, tag="vww")
                nc.vector.tensor_mul(vww[:], vwsp[:], omc[b][:])
                nc.vector.tensor_copy(wv_w[b][:], vww[:])

            def block(b):
                """Per-batch dense compute: G' = x_g M^T, S, exp, Z, num,
                combine."""
                # --- G'^T[d, s] = sum_d' M^T[d', d] xgT[d', s] ---
                Gsb = gbig.tile([P, DC, CAP], BF16, tag="Gsb", name=f"Gsb{b}")
                for dt in range(DC):
                    pgt = psg.tile(
                        [P, 512], F32, tag="g", bufs=3, name=f"G{b}_{dt}"
                    )
                    for dc in range(DC):
                        nc.tensor.matmul(
                            pgt[:, 0:CAP],
                            Msb[:, dc, ts(dt, P)],
                            xgT[b][:, dc],
                            start=(dc == 0),
                            stop=(dc == DC - 1),
                        )
                    nc.scalar.activation(Gsb[:, dt], pgt[:, 0:CAP], AF.Copy)

                # --- S[s, t] = sum_d G'^T[d, s] xgT[d, t] + a[t] (+c[s]/D
                # via exp bias); E = exp(S/D) with free-axis row sums ---
                Eg = gbig.tile([P, SC, CAP], BF16, tag="Eg", name=f"Eg{b}")
                eacc = gsb.tile([P, SC], F32, tag="eacc")
                erw = gsb.tile([P, SC], F32, tag="erw")
                for st in range(SC):
                    pss = psg.tile(
                        [P, 512], F32, tag="g", bufs=3, name=f"S{b}_{st}"
                    )
                    for dt in range(DC):
                        nc.tensor.matmul(
                            pss[:, 0:CAP],
                            Gsb[:, dt, ts(st, P)],
                            xgT[b][:, dt],
                            start=(dt == 0),
                            stop=False,
                        )
                    nc.tensor.matmul(
                        pss[:, 0:CAP],
                        ones_bf[:],
                        a_bf[b][:],
                        start=False,
                        stop=True,
                    )
                    nc.scalar.activation(
                        Eg[:, st], pss[:, 0:CAP], AF.Exp,
                        bias=cspd[b][:, st : st + 1], scale=float(1.0 / D),
                        accum_out=eacc[:, st : st + 1],
                    )
                    nc.vector.tensor_scalar(
                        erw[:, st : st + 1],
                        Eg[:, st, CAP - 1 : CAP],
                        float(T - CAP),
                        eacc[:, st : st + 1],
                        op0=OP.mult,
                        op1=OP.add,
                    )
                # Z = omega . erw (both [slot] in partition-chunk layout)
                scr3 = gsb.tile([P, SC], F32, tag="scr3")
                zp = gsb.tile([P, 1], F32, tag="zp")
                nc.vector.tensor_mul(scr3[:], erw[:], omc[b][:])
                nc.vector.reduce_sum(zp[:], scr3[:], axis=AX.X)
                za = gsb.tile([P, 1], F32, tag="za")
                nc.gpsimd.partition_all_reduce(
                    za[:], zp[:], channels=P, reduce_op=RED.add
                )
                rZ = gsb.tile([P, 1], F32, tag="rZ")
                nc.vector.reciprocal(rZ[:], za[:])
                # num[t] = sum_s E[s, t] (omega_s vw[s])
                numg = gsb.tile([P, SC], F32, tag="numg")
                for ti in range(SC):
                    pn = psg.tile(
                        [P, 512], F32, tag="g", bufs=3, name=f"pn{b}_{ti}"
                    )
                    for sc in range(SC):
                        nc.tensor.matmul(
                            pn[:, 0:1],
                            Eg[:, sc, ts(ti, P)],
                            wv_w[b][:, sc : sc + 1],
                            start=(sc == 0),
                            stop=(sc == SC - 1),
                        )
                    nc.scalar.activation(numg[:, ti : ti + 1], pn[:, 0:1], AF.Copy)
                # out_sum = num/Z + boS; scatter to token space
                outg = gsb.tile([P, SC], F32, tag="outg")
                nc.vector.tensor_scalar(
                    outg[:], numg[:], rZ[:], boS_bc[:], op0=OP.mult, op1=OP.add
                )
                for i in range(SC):
                    nc.gpsimd.indirect_dma_start(
                        out=sc_d[:, None],
                        out_offset=bass.IndirectOffsetOnAxis(
                            ap=idxt[b][:, i : i + 1], axis=0
                        ),
                        in_=outg[:, i : i + 1],
                        in_offset=None,
                        bounds_check=N - 1,
                        oob_is_err=False,
                    )
                scb = gsb.tile([P, ST], F32, tag="scb")
                nc.sync.dma_start(
                    scb[:],
                    sc_d[b * T : (b + 1) * T].rearrange("(c p) -> p c", p=P),
                )
                ob = gsb.tile([P, ST], F32, tag="ob")
                nc.vector.tensor_mul(ob[:], scb[:], cwb[b][:])
                nc.sync.dma_start(out_d.ap()[b], ob[:])

            # ---------------- pipeline ----------------
            wq_sb = wqk.tile([P, DC, D], BF16, tag="wq", name="wq")
            wk_sb = wqk.tile([P, DC, D], BF16, tag="wk", name="wk")
            gate()
            nc.sync.dma_start(
                wq_sb[:], wqT_d.ap().rearrange("(c p) d -> p c d", p=P)
            )
            nc.sync.dma_start(
                wk_sb[:], wkT_d.ap().rearrange("(c p) d -> p c d", p=P)
            )
            # zero the token-space scatter target once
            for bb in range(B):
                nc.sync.dma_start(
                    sc_d[bb * T : (bb + 1) * T].rearrange("(c p) -> p c", p=P),
                    zt[:],
                )
            for b in range(B):
                gather1(b)
            for b in range(B):
                nc.gpsimd.sparse_gather(
                    idxw4[b][:], vals4[b][:], num_found=nfu4[b][:]
                )
            m_build(range(DC))
            for b in range(B):
                gather2(b)
            for b in range(B):
                nc.gpsimd.dma_gather(
                    xgT[b][:],
                    xnb.ap(),
                    idx16[b][:],
                    num_idxs=CAP,
                    num_idxs_reg=CAP,
                    elem_size=D,
                    transpose=True,
                )
            for b in range(B):
                gather3(b)
            block(0)
            block(1)
            block(2)
            block(3)


def build_nc():
    nc = bacc.Bacc("TRN2", target_bir_lowering=False, debug=False, num_devices=8)
    xTs = nc.dram_tensor("xTs", [D, TS], F32, kind="ExternalInput")
    xnb = nc.dram_tensor("xnb", [N + 1, D], BF16, kind="ExternalInput")
    wg_d = nc.dram_tensor("wg", [D, E], F32, kind="ExternalInput")
    wqT_d = nc.dram_tensor("wqT", [DH, D], BF16, kind="ExternalInput")
    wkT_d = nc.dram_tensor("wkT", [DH, D], BF16, kind="ExternalInput")
    w3_d = nc.dram_tensor("w3", [D, 3], BF16, kind="ExternalInput")
    sconst_d = nc.dram_tensor("sconst", [8], F32, kind="ExternalInput")
    out_d = nc.dram_tensor("contrib", [B, P, ST], F32, kind="ExternalOutput")
    with tile.TileContext(nc) as tc:
        _emit(
            nc,
            tc,
            (xTs, xnb, wg_d, wqT_d, wkT_d, w3_d, sconst_d),
            (out_d,),
        )
    nc.compile()
    return nc


def make_in_maps(x, wg, wqkv, bqkv, wo, bo):
    xn = np.ascontiguousarray(x.reshape(N, D), dtype=np.float32)
    xT = np.ascontiguousarray(xn.T)
    wg32 = np.ascontiguousarray(wg, dtype=np.float32)
    xnb = np.zeros((N + 1, D), dtype=ml_dtypes.bfloat16)
    xnb[:N] = xn
    in_maps = []
    for e in range(E):
        wq = wqkv[e][:, 0::3].astype(np.float64)
        wk = wqkv[e][:, 1::3].astype(np.float64)
        wv = wqkv[e][:, 2::3].astype(np.float64)
        bq = bqkv[e][0::3].astype(np.float64)
        bk = bqkv[e][1::3].astype(np.float64)
        bv = bqkv[e][2::3].astype(np.float64)
        wos = wo[e].astype(np.float64).sum(axis=1)
        u = wv @ wos
        aw = wq @ bk
        cv = wk @ bq
        cc = float(bq @ bk)
        c0 = float(bv @ wos)
        boS = float(bo[e].astype(np.float64).sum())
        in_maps.append(
            {
                "xTs": np.ascontiguousarray(xT[:, e * TS : (e + 1) * TS]),
                "xnb": xnb,
                "wg": wg32,
                "wqT": np.ascontiguousarray(wq.T).astype(ml_dtypes.bfloat16),
                "wkT": np.ascontiguousarray(wk.T).astype(ml_dtypes.bfloat16),
                "w3": np.ascontiguousarray(
                    np.stack([aw, cv, u], axis=1)
                ).astype(ml_dtypes.bfloat16),
                "sconst": np.array(
                    [cc, 0.0, c0, boS, 0.0, 0.0, 0.0, 0.0], dtype=np.float32
                ),
            }
        )
    return in_maps


def run_device(in_maps, trace=False):
    if "nc" not in _CACHE:
        _CACHE["nc"] = build_nc()
    return bass_utils.run_bass_kernel_spmd(
        _CACHE["nc"], in_maps, core_ids=list(range(E)), trace=trace
    )


def kernel(x, wg, wqkv, bqkv, wo, bo, top_k):
    assert int(top_k) == 2, f"kernel hardcodes top_k=2, got {top_k}"
    x = np.asarray(x, np.float32)
    wg = np.asarray(wg, np.float32)
    wqkv = np.asarray(wqkv, np.float32)
    bqkv = np.asarray(bqkv, np.float32)
    wo = np.asarray(wo, np.float32)
    bo = np.asarray(bo, np.float32)

    res = run_device(make_in_maps(x, wg, wqkv, bqkv, wo, bo))
    total = np.zeros((B, T), np.float64)
    for c in range(E):
        contrib = res.results[c]["contrib"]  # [B, P, ST], t = tt*128 + p
        total += contrib.transpose(0, 2, 1).reshape(B, T).astype(np.float64)
    m = total.max(axis=1, keepdims=True)
    ls = total - m - np.log(np.exp(total - m).sum(axis=1, keepdims=True))
    return ls.astype(np.float32)


# revision 33
# speedup vs baseline: 1.6752x; 1.0883x over previous
"""MoE-routing attention kernel for 8 Trainium2 NeuronCores — v2.

Expert parallelism (1 expert per core), full inputs in, full output out.
v2 restructures v1 around three ideas:

1. Merged q/k projection: S[s,t] = q_t.k_s = x_t M x_s + a[t] + c[s] + cc
   with M = Wq Wk^T precomputed ON DEVICE once per expert (dense 1024^3
   matmul that also warms the PE), a = x.(Wq bk), c = x.(Wk bq),
   cc = bq.bk. Per batch this is ONE projection G' = x_g M^T (64 MMs)
   plus 24 score MMs instead of two projections (128 MMs) + scores.
2. bf16 everywhere on the S path (validated: rel err stays at 1e-6
   because scores are divided by D=1024 before exp). bf16 halves
   LDWEIGHTS cost (FWL) and DMA volume.
3. Gathered x arrives TRANSPOSED directly via dma_gather(transpose=True)
   (bf16, one instruction per batch) — no PE transposes, no PSUM copies.
   Pad slots point at a host-appended zero row of x.

The gate runs with wg STATIONARY (8 LDWEIGHTS total instead of 256):
logits^T = wg^T @ x^T in [8, N] layout, then 8 tiny PE transposes per
batch back to [token, 8] for the top-2/softmax tail (fp32 - routing
flips are the one numerically fragile spot).

Weight-derived vectors (u = wv @ wo_rowsum, aw = Wq bk, cv = Wk bq and
scalars cc, c0 = bv.wo_rowsum, boS = sum bo) are folded on the host like
bias preprocessing. All model FLOPs (gate, M, G', S, attention combine)
stay on device.

Host: sums the 8 per-core [B,T] contribution vectors and applies the
final log_softmax (the SPMD combine/unshard step), as in v1.
"""

import sys

import numpy as np

for _p in ("/opt/trn_rl_repo", "/root/.axon_site/_ro/trn_rl_repo"):
    if _p not in sys.path:
        sys.path.append(_p)

import ml_dtypes  # noqa: E402

import concourse.bass as bass  # noqa: E402
import concourse.bass_isa as bass_isa  # noqa: E402
import concourse.mybir as mybir  # noqa: E402
import concourse.tile as tile  # noqa: E402
from concourse import bacc  # noqa: E402
from concourse import bass_utils  # noqa: E402
from concourse.bass import ts  # noqa: E402
from concourse.masks import make_identity  # noqa: E402

P = 128
B, T, D, E = 4, 1024, 1024, 8
DH = D
N = B * T
DC = D // P  # 8 contraction chunks
ST = T // P  # 8 token tiles per batch
CAP = 384  # gathered slot capacity per (expert, batch)
SC = CAP // P  # 3 slot tiles
NZ = N  # index of the host-appended all-zero x row (pad target)
BIG = 1 << 20
F32 = mybir.dt.float32
F32R = mybir.dt.float32r
BF16 = mybir.dt.bfloat16
I32 = mybir.dt.int32
I16 = mybir.dt.int16
AF = mybir.ActivationFunctionType
OP = mybir.AluOpType
AX = mybir.AxisListType
RED = bass_isa.ReduceOp

_CACHE = {}


TS = N // E  # tokens per core's gate shard


def _emit(nc, tc, dt_in, dt_out):
    (xTs, xnb, wg_d, wqT_d, wkT_d, w3_d, sconst_d) = dt_in
    (out_d,) = dt_out

    with tc.tile_pool(name="const", bufs=1) as const, tc.tile_pool(
        name="wqk", bufs=1
    ) as wqk, tc.tile_pool(name="msb", bufs=1) as msbp, tc.tile_pool(
        name="drams", bufs=1, space="DRAM"
    ) as dramp:
        # ---------------- constants ----------------
        wg_sb = const.tile([P, DC, E], F32)
        nc.sync.dma_start(wg_sb[:], wg_d.ap().rearrange("(c p) e -> p c e", p=P))
        w3_sb = const.tile([P, DC, 3], BF16)
        nc.sync.dma_start(w3_sb[:], w3_d.ap().rearrange("(c p) k -> p c k", p=P))
        bias3 = const.tile([3, 1], F32)
        nc.sync.dma_start(bias3[:], sconst_d.ap()[0:3, None])
        boS_bc = const.tile([P, 1], F32)
        nc.sync.dma_start(
            boS_bc[:], sconst_d.ap()[3:4][None, :].to_broadcast([P, 1])
        )

        idn = const.tile([P, P], F32)
        make_identity(nc, idn[:])
        ones_bf = const.tile([1, P], BF16)
        nc.vector.memset(ones_bf[:], 1.0)
        iosc_i = const.tile([P, SC], I32)  # value = slot j = c*128 + p
        nc.gpsimd.iota(iosc_i[:], pattern=[[P, SC]], base=0, channel_multiplier=1)
        iosc = const.tile([P, SC], F32)
        nc.vector.tensor_copy(iosc[:], iosc_i[:])
        tk16_i = const.tile([16, T // 16], I32)  # token id t = f*16 + p
        nc.gpsimd.iota(
            tk16_i[:], pattern=[[16, T // 16]], base=0, channel_multiplier=1
        )
        tk16 = const.tile([16, T // 16], F32)
        nc.vector.tensor_copy(tk16[:], tk16_i[:])
        zt = const.tile([P, ST], F32)
        nc.vector.memset(zt[:], 0.0)

        Msb = msbp.tile([P, DC, D], BF16)  # M^T[d', d]; chunk dc = d' rows

        sc_d = dramp.tile([N], F32, tag="scd", name="scd")
        a2a_in = dramp.tile([E, 2, TS], F32, tag="a2ain", name="a2ain")
        a2a_out = dramp.tile([E, 2, TS], F32, tag="a2aout", name="a2aout")
        iw_d = [
            dramp.tile([CAP], F32, tag=f"iwd{b}", name=f"iwd{b}")
            for b in range(B)
        ]
        nf_d = [
            dramp.tile([1], mybir.dt.uint32, tag=f"nfd{b}", name=f"nfd{b}")
            for b in range(B)
        ]
        cu_d = [
            dramp.tile([2, CAP], F32, tag=f"cud{b}", name=f"cud{b}")
            for b in range(B)
        ]

        with tc.tile_pool(name="pb", bufs=1) as pbp, tc.tile_pool(
            name="gsb", bufs=3
        ) as gsb, tc.tile_pool(name="gbig", bufs=2) as gbig, tc.tile_pool(
            name="psm", bufs=1, space="PSUM"
        ) as psm, tc.tile_pool(name="psg", bufs=1, space="PSUM") as psg:
            # per-batch persistent tiles
            cwb = [
                pbp.tile([P, ST], F32, tag=f"cwb{b}", name=f"cwb{b}")
                for b in range(B)
            ]
            idxt = [
                pbp.tile([P, SC], I32, tag=f"idxt{b}", name=f"idxt{b}")
                for b in range(B)
            ]
            idx16 = [
                pbp.tile([P, CAP // 16], I16, tag=f"idx16{b}", name=f"idx16{b}")
                for b in range(B)
            ]
            xgT = [
                pbp.tile([P, DC, CAP], BF16, tag=f"xgT{b}", name=f"xgT{b}")
                for b in range(B)
            ]
            omc = [
                pbp.tile([P, SC], F32, tag=f"omc{b}", name=f"omc{b}")
                for b in range(B)
            ]
            cspd = [
                pbp.tile([P, SC], F32, tag=f"cspd{b}", name=f"cspd{b}")
                for b in range(B)
            ]
            wv_w = [
                pbp.tile([P, SC], BF16, tag=f"wvw{b}", name=f"wvw{b}")
                for b in range(B)
            ]
            a_bf = [
                pbp.tile([1, CAP], BF16, tag=f"abf{b}", name=f"abf{b}")
                for b in range(B)
            ]

            def m_build(fcs):
                """M^T[d'-tile, d] = sum_f WkT[f, d']^T WqT[f, d] for d'-tiles
                in fcs. Each group accumulates over all 8 f chunks."""
                for dt in fcs:
                    for half in range(2):
                        pm = psm.tile(
                            [P, 512], F32, tag="m", bufs=2,
                            name=f"m{dt}_{half}",
                        )
                        for fc in range(DC):
                            nc.tensor.matmul(
                                pm[:],
                                wk_sb[:, fc, ts(dt, P)],
                                wq_sb[:, fc, ts(half, 512)],
                                start=(fc == 0),
                                stop=(fc == DC - 1),
                            )
                        nc.scalar.activation(
                            Msb[:, dt, ts(half, 512)], pm[:], AF.Copy
                        )

            def gate():
                """Sharded gate: this core computes logits for its own
                TS-token slice (full fp32), top-2 + softmax for ALL experts
                in [8, TS] layout, then AllToAll so every core ends up with
                its own expert's mask/cw over all N tokens."""
                xts = pbp.tile([P, DC, TS], F32, tag="xts", name="xts")
                nc.sync.dma_start(
                    xts[:], xTs.ap().rearrange("(c p) n -> p c n", p=P)
                )
                pg = psm.tile([P, 512], F32, tag="m", bufs=2, name="pgate")
                for dc in range(DC):
                    nc.tensor.matmul(
                        pg[0:E, 0:TS],
                        wg_sb[:, dc],
                        xts[:, dc],
                        start=(dc == 0),
                        stop=(dc == DC - 1),
                    )
                lsh = gsb.tile([E, TS], F32, tag="lsh")
                nc.scalar.activation(lsh[:], pg[0:E, 0:TS], AF.Copy)
                # transpose each 128-token tile to [token, E] and run the
                # top-2/softmax tail for ALL experts (PE/DVE/ACT only)
                mk4 = gsb.tile([P, TS // P, E], F32, tag="mk4")
                cw4 = gsb.tile([P, TS // P, E], F32, tag="cw4")
                for tt in range(TS // P):
                    tp = psg.tile(
                        [P, 512], F32, tag="g", bufs=3, name=f"tp{tt}"
                    )
                    nc.tensor.transpose(
                        tp[:, 0:E], lsh[:, ts(tt, P)], idn[0:E, 0:E]
                    )
                    gl = gsb.tile([P, E], F32, tag="gl")
                    nc.scalar.activation(gl[:], tp[:, 0:E], AF.Copy)
                    mx8 = gsb.tile([P, 8], F32, tag="mx8")
                    nc.vector.max(out=mx8[:], in_=gl[:])
                    mxn = gsb.tile([P, 1], F32, tag="mxn")
                    nc.vector.tensor_scalar_mul(mxn[:], mx8[:, 0:1], -1.0)
                    probs = gsb.tile([P, E], F32, tag="probs")
                    se = gsb.tile([P, 1], F32, tag="se")
                    nc.scalar.activation(
                        probs[:], gl[:], AF.Exp, bias=mxn[:], scale=1.0,
                        accum_out=se[:],
                    )
                    rs = gsb.tile([P, 1], F32, tag="rs")
                    nc.vector.reciprocal(rs[:], se[:])
                    nc.vector.tensor_scalar(
                        mk4[:, tt, :], gl[:], mx8[:, 1:2], None, op0=OP.is_ge
                    )
                    nc.vector.scalar_tensor_tensor(
                        cw4[:, tt, :], probs[:], rs[:], mk4[:, tt, :],
                        op0=OP.mult, op1=OP.mult,
                    )
                # transpose back to [expert, token] for the AllToAll pack
                mkE = gsb.tile([E, TS], F32, tag="mkE")
                cwE = gsb.tile([E, TS], F32, tag="cwE")
                for tt in range(TS // P):
                    for src, dst in ((mk4, mkE), (cw4, cwE)):
                        tq = psg.tile(
                            [P, 512], F32, tag="g", bufs=3,
                            name=f"tq{tt}_{dst.name}",
                        )
                        nc.tensor.transpose(
                            tq[0:E, 0:P], src[:, tt, :], idn[:]
                        )
                        nc.scalar.activation(
                            dst[:, ts(tt, P)], tq[0:E, 0:P], AF.Copy
                        )
                nc.sync.dma_start(a2a_in[:, 0, :], mkE[:])
                nc.sync.dma_start(a2a_in[:, 1, :], cwE[:])
                nc.gpsimd.collective_compute(
                    "AllToAll",
                    mybir.AluOpType.bypass,
                    replica_groups=[list(range(E))],
                    ins=[a2a_in[:]],
                    outs=[a2a_out[:]],
                )
                for b in range(B):
                    for r in range(2):
                        nc.sync.dma_start(
                            cwb[b][:, 4 * r : 4 * (r + 1)],
                            a2a_out[2 * b + r, 1, :].rearrange(
                                "(c p) -> p c", p=P
                            ),
                        )

            vals4 = [
                pbp.tile(
                    [16, T // 16 + SC * 8], F32, tag=f"vals{b}", name=f"vals{b}"
                )
                for b in range(B)
            ]
            idxw4 = [
                pbp.tile([16, CAP // 16], F32, tag=f"idxw{b}", name=f"idxw{b}")
                for b in range(B)
            ]
            nfu4 = [
                pbp.tile([1, 1], mybir.dt.uint32, tag=f"nfu{b}", name=f"nfu{b}")
                for b in range(B)
            ]
            cf4 = [
                pbp.tile([P, 1], F32, tag=f"cf{b}", name=f"cf{b}")
                for b in range(B)
            ]

            def gather1(b):
                """Per-token values (global token id if routed else -1) in a
                16-partition wrap, with CAP appended entries pointing at the
                zero x row so pads compact in behind the routed tokens."""
                vals = vals4[b]
                nc.vector.memset(vals[:, T // 16 :], float(NZ))
                for r in range(2):
                    nc.sync.dma_start(
                        vals[:, 32 * r : 32 * (r + 1)],
                        a2a_out[2 * b + r, 0, :].rearrange(
                            "(f p) -> p f", p=16
                        ),
                    )
                t1 = gsb.tile([16, T // 16], F32, tag="t1")
                nc.vector.tensor_scalar(
                    t1[:], tk16[:], float(b * T + 1), None, op0=OP.add
                )
                nc.vector.tensor_mul(t1[:], t1[:], vals[:, 0 : T // 16])
                nc.vector.tensor_scalar(
                    vals[:, 0 : T // 16], t1[:], -1.0, None, op0=OP.add
                )

            def gather2(b):
                """cf = C, int16 idx list + per-group replicas, slot->token
                int32 list via a DRAM bounce."""
                nc.sync.dma_start(nf_d[b][:, None], nfu4[b][:])
                nfb = gsb.tile([P, 1], mybir.dt.uint32, tag="nfb")
                nc.sync.dma_start(
                    nfb[:], nf_d[b][None, :].to_broadcast([P, 1])
                )
                nc.vector.tensor_copy(cf4[b][:], nfb[:])
                nc.vector.tensor_scalar(
                    cf4[b][:], cf4[b][:], float(-CAP), None, op0=OP.add
                )
                nc.vector.tensor_copy(idx16[b][0:16, :], idxw4[b][:])
                for r in range(1, P // 16):
                    eng = nc.sync if r % 2 == 0 else nc.scalar
                    eng.dma_start(
                        idx16[b][16 * r : 16 * (r + 1), :], idx16[b][0:16, :]
                    )
                nc.sync.dma_start(
                    iw_d[b].rearrange("(f p) -> p f", p=16), idxw4[b][:]
                )
                idxtf = gsb.tile([P, SC], F32, tag="idxtf")
                nc.sync.dma_start(
                    idxtf[:], iw_d[b].rearrange("(c p) -> p c", p=P)
                )
                nc.vector.tensor_copy(idxt[b][:], idxtf[:])

            def gather3(b):
                """omega weights and the a/c/vw vectors."""
                cf = cf4[b]
                # omega: 1 for j < C, (T - C) at j == CAP-1, else 0
                tmc = gsb.tile([P, 1], F32, tag="tmc")
                nc.vector.tensor_scalar(
                    tmc[:], cf[:], -1.0, float(T), op0=OP.mult, op1=OP.add
                )
                rep = gsb.tile([P, SC], F32, tag="rep")
                nc.vector.tensor_scalar(
                    omc[b][:], iosc[:], cf[:], None, op0=OP.is_lt
                )
                nc.vector.tensor_scalar(
                    rep[:], iosc[:], float(CAP - 1), None, op0=OP.is_equal
                )
                nc.vector.tensor_scalar(rep[:], rep[:], tmc[:], None, op0=OP.mult)
                nc.vector.tensor_add(omc[b][:], omc[b][:], rep[:])

                # --- a, c, vw vectors: [3, CAP] = w3^T x_g^T (+ biases) ---
                p3 = psg.tile([P, 512], F32, tag="g", bufs=3, name=f"p3{b}")
                for dc in range(DC):
                    nc.tensor.matmul(
                        p3[0:3, 0:CAP],
                        w3_sb[:, dc],
                        xgT[b][:, dc],
                        start=(dc == 0),
                        stop=(dc == DC - 1),
                    )
                avc = gsb.tile([3, CAP], F32, tag="avc")
                nc.scalar.activation(
                    avc[:], p3[0:3, 0:CAP], AF.Identity, bias=bias3[:]
                )
                nc.vector.tensor_copy(a_bf[b][:], avc[0:1, :])
                nc.sync.dma_start(cu_d[b][:], avc[1:3, :])
                nc.sync.dma_start(
                    cspd[b][:], cu_d[b][0].rearrange("(c p) -> p c", p=P)
                )
                vwsp = gsb.tile([P, SC], F32, tag="vwsp")
                nc.sync.dma_start(
                    vwsp[:], cu_d[b][1].rearrange("(c p) -> p c", p=P)
                )
                nc.vector.tensor_scalar_mul(cspd[b][:], cspd[b][:], 1.0 / D)
                vww = gsb.tile([P, SC], F32, tag="vww")
                nc.vector.tensor_mul(vww[:], vwsp[:], omc[b][:])
                nc.vector.tensor_copy(wv_w[b][:], vww[:])

            def block(b):
                """Per-batch dense compute: G' = x_g M^T, S, exp, Z, num,
                combine."""
                # --- G'^T[d, s] = sum_d' M^T[d', d] xgT[d', s] ---
                Gsb = gbig.tile([P, DC, CAP], BF16, tag="Gsb", name=f"Gsb{b}")
                for dt in range(DC):
                    pgt = psg.tile(
                        [P, 512], F32, tag="g", bufs=3, name=f"G{b}_{dt}"
                    )
                    for dc in range(DC):
                        nc.tensor.matmul(
                            pgt[:, 0:CAP],
                            Msb[:, dc, ts(dt, P)],
                            xgT[b][:, dc],
                            start=(dc == 0),
                            stop=(dc == DC - 1),
                        )
                    nc.scalar.activation(Gsb[:, dt], pgt[:, 0:CAP], AF.Copy)

                # --- S[s, t] = sum_d G'^T[d, s] xgT[d, t] + a[t] (+c[s]/D
                # via exp bias); E = exp(S/D) with free-axis row sums ---
                Eg = gbig.tile([P, SC, CAP], BF16, tag="Eg", name=f"Eg{b}")
                eacc = gsb.tile([P, SC], F32, tag="eacc")
                erw = gsb.tile([P, SC], F32, tag="erw")
                for st in range(SC):
                    pss = psg.tile(
                        [P, 512], F32, tag="g", bufs=3, name=f"S{b}_{st}"
                    )
                    for dt in range(DC):
                        nc.tensor.matmul(
                            pss[:, 0:CAP],
                            Gsb[:, dt, ts(st, P)],
                            xgT[b][:, dt],
                            start=(dt == 0),
                            stop=False,
                        )
                    nc.tensor.matmul(
                        pss[:, 0:CAP],
                        ones_bf[:],
                        a_bf[b][:],
                        start=False,
                        stop=True,
                    )
                    nc.scalar.activation(
                        Eg[:, st], pss[:, 0:CAP], AF.Exp,
                        bias=cspd[b][:, st : st + 1], scale=float(1.0 / D),
                        accum_out=eacc[:, st : st + 1],
                    )
                    nc.vector.tensor_scalar(
                        erw[:, st : st + 1],
                        Eg[:, st, CAP - 1 : CAP],
                        float(T - CAP),
                        eacc[:, st : st + 1],
                        op0=OP.mult,
                        op1=OP.add,
                    )
                # Z = omega . erw (both [slot] in partition-chunk layout)
                scr3 = gsb.tile([P, SC], F32, tag="scr3")
                zp = gsb.tile([P, 1], F32, tag="zp")
                nc.vector.tensor_mul(scr3[:], erw[:], omc[b][:])
                nc.vector.reduce_sum(zp[:], scr3[:], axis=AX.X)
                za = gsb.tile([P, 1], F32, tag="za")
                nc.gpsimd.partition_all_reduce(
                    za[:], zp[:], channels=P, reduce_op=RED.add
                )
                rZ = gsb.tile([P, 1], F32, tag="rZ")
                nc.vector.reciprocal(rZ[:], za[:])
                # num[t] = sum_s E[s, t] (omega_s vw[s])
                numg = gsb.tile([P, SC], F32, tag="numg")
                for ti in range(SC):
                    pn = psg.tile(
                        [P, 512], F32, tag="g", bufs=3, name=f"pn{b}_{ti}"
                    )
                    for sc in range(SC):
                        nc.tensor.matmul(
                            pn[:, 0:1],
                            Eg[:, sc, ts(ti, P)],
                            wv_w[b][:, sc : sc + 1],
                            start=(sc == 0),
                            stop=(sc == SC - 1),
                        )
                    nc.scalar.activation(numg[:, ti : ti + 1], pn[:, 0:1], AF.Copy)
                # out_sum = num/Z + boS; scatter to token space
                outg = gsb.tile([P, SC], F32, tag="outg")
                nc.vector.tensor_scalar(
                    outg[:], numg[:], rZ[:], boS_bc[:], op0=OP.mult, op1=OP.add
                )
                for i in range(SC):
                    nc.gpsimd.indirect_dma_start(
                        out=sc_d[:, None],
                        out_offset=bass.IndirectOffsetOnAxis(
                            ap=idxt[b][:, i : i + 1], axis=0
                        ),
                        in_=outg[:, i : i + 1],
                        in_offset=None,
                        bounds_check=N - 1,
                        oob_is_err=False,
                    )
                scb = gsb.tile([P, ST], F32, tag="scb")
                nc.sync.dma_start(
                    scb[:],
                    sc_d[b * T : (b + 1) * T].rearrange("(c p) -> p c", p=P),
                )
                ob = gsb.tile([P, ST], F32, tag="ob")
                nc.vector.tensor_mul(ob[:], scb[:], cwb[b][:])
                nc.sync.dma_start(out_d.ap()[b], ob[:])

            # ---------------- pipeline ----------------
            wq_sb = wqk.tile([P, DC, D], BF16, tag="wq", name="wq")
            wk_sb = wqk.tile([P, DC, D], BF16, tag="wk", name="wk")
            gate()
            nc.sync.dma_start(
                wq_sb[:], wqT_d.ap().rearrange("(c p) d -> p c d", p=P)
            )
            nc.sync.dma_start(
                wk_sb[:], wkT_d.ap().rearrange("(c p) d -> p c d", p=P)
            )
            # zero the token-space scatter target once
            for bb in range(B):
                nc.sync.dma_start(
                    sc_d[bb * T : (bb + 1) * T].rearrange("(c p) -> p c", p=P),
                    zt[:],
                )
            for b in range(B):
                gather1(b)
            for b in range(B):
                nc.gpsimd.sparse_gather(
                    idxw4[b][:], vals4[b][:], num_found=nfu4[b][:]
                )
            m_build(range(DC))
            for b in range(B):
                gather2(b)
            for b in range(B):
                nc.gpsimd.dma_gather(
                    xgT[b][:],
                    xnb.ap(),
                    idx16[b][:],
                    num_idxs=CAP,
                    num_idxs_reg=CAP,
                    elem_size=D,
                    transpose=True,
                )
            for b in range(B):
                gather3(b)
            block(0)
            block(1)
            block(2)
            block(3)


def build_nc():
    nc = bacc.Bacc("TRN2", target_bir_lowering=False, debug=False, num_devices=8)
    xTs = nc.dram_tensor("xTs", [D, TS], F32, kind="ExternalInput")
    xnb = nc.dram_tensor("xnb", [N + 1, D], BF16, kind="ExternalInput")
    wg_d = nc.dram_tensor("wg", [D, E], F32, kind="ExternalInput")
    wqT_d = nc.dram_tensor("wqT", [DH, D], BF16, kind="ExternalInput")
    wkT_d = nc.dram_tensor("wkT", [DH, D], BF16, kind="ExternalInput")
    w3_d = nc.dram_tensor("w3", [D, 3], BF16, kind="ExternalInput")
    sconst_d = nc.dram_tensor("sconst", [8], F32, kind="ExternalInput")
    out_d = nc.dram_tensor("contrib", [B, P, ST], F32, kind="ExternalOutput")
    with tile.TileContext(nc) as tc:
        _emit(
            nc,
            tc,
            (xTs, xnb, wg_d, wqT_d, wkT_d, w3_d, sconst_d),
            (out_d,),
        )
    nc.compile()
    return nc


def make_in_maps(x, wg, wqkv, bqkv, wo, bo):
    xn = np.ascontiguousarray(x.reshape(N, D), dtype=np.float32)
    xT = np.ascontiguousarray(xn.T)
    wg32 = np.ascontiguousarray(wg, dtype=np.float32)
    xnb = np.zeros((N + 1, D), dtype=ml_dtypes.bfloat16)
    xnb[:N] = xn
    in_maps = []
    for e in range(E):
        wq = wqkv[e][:, 0::3].astype(np.float64)
        wk = wqkv[e][:, 1::3].astype(np.float64)
        wv = wqkv[e][:, 2::3].astype(np.float64)
        bq = bqkv[e][0::3].astype(np.float64)
        bk = bqkv[e][1::3].astype(np.float64)
        bv = bqkv[e][2::3].astype(np.float64)
        wos = wo[e].astype(np.float64).sum(axis=1)
        u = wv @ wos
        aw = wq @ bk
        cv = wk @ bq
        cc = float(bq @ bk)
        c0 = float(bv @ wos)
        boS = float(bo[e].astype(np.float64).sum())
        in_maps.append(
            {
                "xTs": np.ascontiguousarray(xT[:, e * TS : (e + 1) * TS]),
                "xnb": xnb,
                "wg": wg32,
                "wqT": np.ascontiguousarray(wq.T).astype(ml_dtypes.bfloat16),
                "wkT": np.ascontiguousarray(wk.T).astype(ml_dtypes.bfloat16),
                "w3": np.ascontiguousarray(
                    np.stack([aw, cv, u], axis=1)
                ).astype(ml_dtypes.bfloat16),
                "sconst": np.array(
                    [cc, 0.0, c0, boS, 0.0, 0.0, 0.0, 0.0], dtype=np.float32
                ),
            }
        )
    return in_maps


def run_device(in_maps, trace=False):
    if "nc" not in _CACHE:
        _CACHE["nc"] = build_nc()
    return bass_utils.run_bass_kernel_spmd(
        _CACHE["nc"], in_maps, core_ids=list(range(E)), trace=trace
    )


def kernel(x, wg, wqkv, bqkv, wo, bo, top_k):
    assert int(top_k) == 2, f"kernel hardcodes top_k=2, got {top_k}"
    x = np.asarray(x, np.float32)
    wg = np.asarray(wg, np.float32)
    wqkv = np.asarray(wqkv, np.float32)
    bqkv = np.asarray(bqkv, np.float32)
    wo = np.asarray(wo, np.float32)
    bo = np.asarray(bo, np.float32)

    res = run_device(make_in_maps(x, wg, wqkv, bqkv, wo, bo))
    total = np.zeros((B, T), np.float64)
    for c in range(E):
        contrib = res.results[c]["contrib"]  # [B, P, ST], t = tt*128 + p
        total += contrib.transpose(0, 2, 1).reshape(B, T).astype(np.float64)
    m = total.max(axis=1, keepdims=True)
    ls = total - m - np.log(np.exp(total - m).sum(axis=1, keepdims=True))
    return ls.astype(np.float32)
